# revision 3
# baseline (speedup 1.0000x reference)
"""AxialAttention TRN2 Bass kernel.

Shapes (hardcoded): x [B=4,T=16,C=256,H=64,W=64] fp32.
N = B*T*H = 4096 lines of [L=64, C=256]; heads=8, d=32.
Sharding: 64 (b,t) blocks -> 8 per core across 8 cores.

Per-core dataflow, per (b,t) block (xT = x[b,t] viewed [C=256, HW=4096],
features-on-partitions native layout):
  qkT  = w_qk^T @ xT          (fp32r MMs, N=512)   q pre-scaled by 1/sqrt(d)
  v    = xT^T @ w_v           (row-major v, lhsT = xT columns)
  per 8-line group, per psum bank b (heads b, b+4 stacked in partitions):
    scoresT[k,q] psum = biasT (via bias-MM: lhsT=bias_h, rhs=tiled-I)
                      + kT_h^T @ qT_h      (bf16, tile_position packed)
    probsT = exp(scoresT)  (ACT, -> bf16 sbuf)     [no max-sub: |s| small]
    Z      = Ez^T @ probsT (column sums, psum [8, 512])
    rz     = 1/Z           (DVE)
    rbc    = Ebc^T @ rz    (broadcast rz over d=32 partitions, psum)
    oT     = v_lh^T @ probsT_lh  (bf16 packed MMs, psum)
    oT_sb  = oT * rbc      (DVE, psum x psum -> sbuf)
    outT   = w_proj^T @ oT_sb + b'  (fp32r MMs; b' = b_v@w_proj + b_proj)
  b_k dropped (softmax shift-invariant); b_q folded into q evacuation bias;
  b_v folded into b'.
"""

import numpy as np

B, T, C, H, W = 4, 16, 256, 64, 64
HEADS, D = 8, 32
NBT = B * T            # 64 (b,t) blocks
NCORES = 8
BT_PER_CORE = NBT // NCORES  # 8
HW = H * W             # 4096 positions per block
L = W                  # 64
GRP = 8                # lines per attention group
NGRP = H // GRP        # 8 groups per block
GQ = GRP * L           # 512 free columns per group


def _build_bass():
    import concourse.bacc as bacc
    import concourse.mybir as mybir
    from concourse.tile import TileContext

    f32 = mybir.dt.float32
    f32r = mybir.dt.float32r
    bf16 = mybir.dt.bfloat16
    AF = mybir.ActivationFunctionType

    nc = bacc.Bacc("TRN2", target_bir_lowering=False, debug=False,
                   num_devices=NCORES)

    # DRAM I/O (per-core shard)
    x_d = nc.dram_tensor("x", [BT_PER_CORE, C, HW], f32r, kind="ExternalInput").ap()
    st_d = nc.dram_tensor("statics", [128, 3108], f32r, kind="ExternalInput").ap()
    out_d = nc.dram_tensor("out", [BT_PER_CORE, C, HW], f32, kind="ExternalOutput").ap()

    def r(ap):
        return ap

    with TileContext(nc) as tc:
        with (
            tc.tile_pool(name="static", bufs=1) as stat,
            tc.tile_pool(name="xt", bufs=2) as pxt,
            tc.tile_pool(name="qk", bufs=4) as pqk,
            tc.tile_pool(name="vsb", bufs=2) as pv,
            tc.tile_pool(name="probs", bufs=3) as ppr,
            tc.tile_pool(name="zsb", bufs=3) as pz,
            tc.tile_pool(name="osb", bufs=6) as po,
            tc.tile_pool(name="outsb", bufs=4) as pout,
            tc.tile_pool(name="psA", bufs=4, space="PSUM") as psA,
            tc.tile_pool(name="psZ", bufs=1, space="PSUM") as psZ,
            tc.tile_pool(name="psM", bufs=3, space="PSUM") as psM,
        ):
            # ---- static loads: one packed DMA ----
            st = stat.tile([128, 3108], f32r, tag="st", name="statics_sb")
            nc.sync.dma_start(out=st, in_=st_d)
            wqk = [st[:, 512 * i:512 * (i + 1)] for i in range(2)]
            wv = [st[:, 1024 + 256 * i:1024 + 256 * (i + 1)] for i in range(2)]
            wp = [[st[:, 1536 + 256 * i + 128 * j:1536 + 256 * i + 128 * (j + 1)]
                   for j in range(2)] for i in range(2)]
            bias_st = st[:, 2048:2304]
            i8t = st[:, 2304:2816]
            bq = st[:, 2816:2818].bitcast(f32)
            bp = st[:, 2818:2820].bitcast(f32)
            ebc = st[0:8, 2820:3076]
            ez = stat.tile([128, 32], bf16, tag="ez", name="ez")
            bias_bf = stat.tile([128, 256], bf16, tag="biasbf", name="bias_bf")
            i8_bf = stat.tile([128, 512], bf16, tag="i8bf", name="i8_bf")
            with nc.allow_low_precision(reason="exact 0/1 constants"):
                nc.vector.tensor_copy(ez, st[:, 3076:3108])
                nc.vector.tensor_copy(bias_bf, bias_st)
                nc.vector.tensor_copy(i8_bf, i8t)

            for bt in range(BT_PER_CORE):
                # ---- load xT ----
                xt = [pxt.tile([128, HW], f32r, tag="xt", name="xt") for _ in range(2)]
                for kc in range(2):
                    nc.sync.dma_start(out=xt[kc], in_=x_d[bt, 128 * kc:128 * (kc + 1), :])

                # ---- qk projection: qkT [512, 4096] -> bf16 sbuf ----
                # feature chunks: mc 0,1 = q (256), mc 2,3 = k (256)
                qkT = [pqk.tile([128, HW], bf16, tag="qkT", name="qkT") for _ in range(4)]
                for mc in range(4):
                    for nn in range(8):
                        ps = psM.tile([128, 512], f32, tag="mm", name="psmm")
                        for kc in range(2):
                            nc.tensor.matmul(
                                ps, r(wqk[kc][:, 128 * mc:128 * (mc + 1)]),
                                r(xt[kc][:, 512 * nn:512 * (nn + 1)]),
                                start=(kc == 0), stop=(kc == 1))
                        dst = qkT[mc][:, 512 * nn:512 * (nn + 1)]
                        if mc < 2:  # q: fold b_q (pre-scaled) per-partition
                            nc.scalar.activation(dst, ps, AF.Identity,
                                                 bias=bq[:, mc:mc + 1], scale=1.0)
                        else:       # k: plain copy (b_k dropped)
                            nc.vector.tensor_copy(dst, ps)

                # ---- v projection (row-major): v [4096, 256] bf16 ----
                # v_sb [128 = 2 lines, 32 chunks * 256]
                v_sb = pv.tile([128, 8192], bf16, tag="vsb", name="vsb")
                for pc in range(32):  # position chunks of 128 (2 lines)
                    ps = psM.tile([128, 256], f32, tag="mm", name="psmmv")
                    for kc in range(2):
                        nc.tensor.matmul(
                            ps, r(xt[kc][:, 128 * pc:128 * (pc + 1)]),
                            r(wv[kc]), start=(kc == 0), stop=(kc == 1))
                    if pc % 2 == 0:
                        nc.scalar.copy(v_sb[:, 256 * pc:256 * (pc + 1)], ps)
                    else:
                        nc.vector.tensor_copy(v_sb[:, 256 * pc:256 * (pc + 1)], ps)

                # partition-swapped v copy so attnv lhsT can start at 64*(h//4)
                v_sw = pv.tile([128, 8192], bf16, tag="vsw", name="vsw")
                nc.sync.dma_start(out=v_sw[0:64, :], in_=v_sb[64:128, :])
                nc.sync.dma_start(out=v_sw[64:128, :], in_=v_sb[0:64, :])

                # ---- attention per 8-line group ----
                for g in range(NGRP):
                    l0 = g * GRP
                    # scoresT psum: 4 banks, bank b = heads (b, b+4)
                    sps = [psA.tile([128, GQ], f32, tag="att", name="psatt") for _ in range(4)]
                    # bias seed MMs (fp32r, N=512)
                    for b in range(4):
                        for hh in range(2):  # h = b + 4*hh
                            nc.tensor.matmul(
                                sps[b][64 * hh:64 * (hh + 1), :],
                                bias_bf[64 * hh:64 * (hh + 1),
                                        64 * b:64 * (b + 1)],
                                i8_bf[64 * hh:64 * (hh + 1), :],
                                start=True, stop=False,
                                tile_position=(64 * hh, 64 * hh))
                    # scoresT accumulate: kT_h^T @ qT_h  (bf16)
                    for li in range(GRP):
                        l = l0 + li
                        for h in range(HEADS):
                            hc, hr = h // 4, h % 4
                            kt = qkT[2 + hc][32 * hr:32 * (hr + 1),
                                             64 * l:64 * (l + 1)]
                            qt = qkT[hc][32 * hr:32 * (hr + 1),
                                         64 * l:64 * (l + 1)]
                            nc.tensor.matmul(
                                sps[hr][64 * hc:64 * (hc + 1),
                                        64 * li:64 * (li + 1)],
                                kt, qt, start=False, stop=True,
                                tile_position=(32 * hr, 64 * hc))
                    # exp -> probsT bf16 sbuf [128, 4*512]
                    probs = ppr.tile([128, 4 * GQ], bf16, tag="probs", name="probs")
                    for b in range(4):
                        nc.scalar.activation(
                            probs[:, GQ * b:GQ * (b + 1)], sps[b], AF.Exp,
                            scale=1.0)
                    # Z: column sums -> psum_z [8, 512]
                    zps = psZ.tile([8, GQ], f32, tag="z", name="psz")
                    for b in range(4):
                        nc.tensor.matmul(
                            zps, ez[:, 8 * b:8 * (b + 1)],
                            probs[:, GQ * b:GQ * (b + 1)],
                            start=(b == 0), stop=(b == 3))
                    z_sb = pz.tile([8, GQ], f32r, tag="z", name="zsb")
                    with nc.allow_low_precision(reason="f32r bits are f32"):
                        nc.vector.reciprocal(z_sb, zps)
                    # broadcast recip over d=32 partitions: rbc [128, 512] x2
                    rbc = [psA.tile([128, GQ], f32, tag="att", name="psatt") for _ in range(2)]
                    rbc_sb = [po.tile([128, GQ], f32, tag="rbc", name="rbcsb")
                              for _ in range(2)]
                    for c in range(2):
                        nc.tensor.matmul(
                            rbc[c], r(ebc[:, 128 * c:128 * (c + 1)]),
                            r(z_sb), start=True, stop=True)
                        nc.scalar.copy(rbc_sb[c], rbc[c])
                    # attn @ v -> oT psum [128, 512] x2 (chunk c = heads 4c..4c+3)
                    ops = [psA.tile([128, GQ], f32, tag="att", name="psatt") for _ in range(2)]
                    for li in range(GRP):
                        l = l0 + li
                        vcol = 256 * (l // 2)
                        for h in range(HEADS):
                            hc, hr = h // 4, h % 4
                            vsrc = v_sb if (l % 2) == hc else v_sw
                            vt = vsrc[64 * hc:64 * (hc + 1),
                                      vcol + 32 * h:vcol + 32 * (h + 1)]
                            pt = probs[64 * hc:64 * (hc + 1),
                                       GQ * hr + 64 * li:GQ * hr + 64 * (li + 1)]
                            nc.tensor.matmul(
                                ops[hc][32 * hr:32 * (hr + 1),
                                        64 * li:64 * (li + 1)],
                                vt, pt, start=True, stop=True,
                                tile_position=(64 * hc, 32 * hr))
                    # oT * rbc -> sbuf f32
                    oT = [po.tile([128, GQ], f32r, tag="oT", name="oT") for _ in range(2)]
                    with nc.allow_low_precision(reason="f32r bits are f32"):
                        for c in range(2):
                            nc.vector.tensor_mul(oT[c], ops[c], rbc_sb[c])
                    # proj + bias -> out sbuf -> DRAM
                    for mc in range(2):
                        ps = psM.tile([128, GQ], f32, tag="mm", name="psproj")
                        for kc in range(2):
                            nc.tensor.matmul(ps, r(wp[kc][mc]), r(oT[kc]),
                                             start=(kc == 0), stop=(kc == 1))
                        osb = pout.tile([128, GQ], f32, tag="out", name="outsb")
                        nc.scalar.activation(osb, ps, AF.Identity,
                                             bias=bp[:, mc:mc + 1], scale=1.0)
                        nc.sync.dma_start(
                            out=out_d[bt, 128 * mc:128 * (mc + 1),
                                      GQ * g:GQ * (g + 1)],
                            in_=osb)
    nc.compile()
    return nc


def _host_inputs(x, relative_bias, w_qkv, b_qkv, w_proj, b_proj):
    import ml_dtypes
    scale = D ** -0.5
    wq = w_qkv[:, :C] * scale          # [256, 256]
    wk = w_qkv[:, C:2 * C]
    wv = w_qkv[:, 2 * C:]
    bqv = b_qkv[:C] * scale            # [256]
    bv = b_qkv[2 * C:]
    wqk_full = np.concatenate([wq, wk], axis=1)        # [256, 512]
    wqk = np.stack([wqk_full[:128], wqk_full[128:]]).astype(np.float32)
    wvs = np.stack([wv[:128], wv[128:]]).astype(np.float32)
    wp = np.zeros((2, 2, 128, 128), np.float32)
    for kc in range(2):
        for mc in range(2):
            wp[kc, mc] = w_proj[128 * kc:128 * (kc + 1),
                                128 * mc:128 * (mc + 1)]
    bq = np.stack([bqv[:128], bqv[128:]], axis=1).astype(np.float32)  # [128,2]
    bpv = bv @ w_proj + b_proj                                       # [256]
    bp = np.stack([bpv[:128], bpv[128:]], axis=1).astype(np.float32)
    bias_st = np.zeros((128, 256), np.float32)
    for h in range(HEADS):
        # lhsT = bias_h [q-contract, k-M]; head h -> rows 64*(h//4), cols 64*(h%4)
        bias_st[64 * (h // 4):64 * (h // 4) + 64,
                64 * (h % 4):64 * (h % 4) + 64] = relative_bias[h]
    i8h = np.tile(np.eye(64, dtype=np.float32), (1, 8))              # [64, 512]
    i8t = np.concatenate([i8h, i8h], axis=0)                         # [128, 512]
    ez = np.zeros((128, 32), np.float32)
    for b in range(4):
        ez[0:64, 8 * b + 2 * b] = 1      # head b    -> z row 2b
        ez[64:128, 8 * b + 2 * b + 1] = 1  # head b+4 -> z row 2b+1
    ebc = np.zeros((8, 256), np.float32)
    for c in range(2):
        for hr in range(4):
            h = 4 * c + hr
            zrow = 2 * (h % 4) + (h // 4)
            ebc[zrow, 128 * c + 32 * hr:128 * c + 32 * (hr + 1)] = 1.0
    st = np.zeros((128, 3108), np.float32)
    st[:, 0:512] = wqk[0]
    st[:, 512:1024] = wqk[1]
    st[:, 1024:1280] = wvs[0]
    st[:, 1280:1536] = wvs[1]
    for i in range(2):
        for j in range(2):
            st[:, 1536 + 256 * i + 128 * j:1536 + 256 * i + 128 * (j + 1)] = wp[i, j]
    st[:, 2048:2304] = bias_st
    st[:, 2304:2816] = i8t
    st[:, 2816:2818] = bq
    st[:, 2818:2820] = bp
    st[0:8, 2820:3076] = ebc
    st[:, 3076:3108] = ez.astype(np.float32)
    return dict(statics=st)


LAST_RESULTS = None


def kernel(x, relative_bias, w_qkv, b_qkv, w_proj, b_proj):
    import os
    import sys
    if '/opt/trn_rl_repo' not in sys.path:
        sys.path.insert(0, '/opt/trn_rl_repo')
    from concourse.bass_utils import run_bass_kernel_spmd

    x = np.asarray(x, np.float32)
    const = _host_inputs(np.asarray(x, np.float32),
                         np.asarray(relative_bias, np.float32),
                         np.asarray(w_qkv, np.float32),
                         np.asarray(b_qkv, np.float32),
                         np.asarray(w_proj, np.float32),
                         np.asarray(b_proj, np.float32))
    # x [B,T,C,H,W] -> [64, 256, 4096]
    xr = np.ascontiguousarray(x.reshape(NBT, C, HW))
    nc = _build_bass()
    in_maps = []
    for c in range(NCORES):
        m = dict(const)
        m["x"] = np.ascontiguousarray(xr[c * BT_PER_CORE:(c + 1) * BT_PER_CORE])
        in_maps.append(m)
    res = run_bass_kernel_spmd(nc, in_maps, list(range(NCORES)),
                               tmpdir=os.environ.get("BASS_TMPDIR"))
    global LAST_RESULTS
    LAST_RESULTS = res
    outs = res.results
    out = np.concatenate([o["out"].reshape(BT_PER_CORE, C, HW) for o in outs],
                         axis=0)
    return out.reshape(B, T, C, H, W).astype(np.float32)



# revision 4
# speedup vs baseline: 1.2586x; 1.2586x over previous
"""AxialAttention TRN2 Bass kernel — v2.

v1 + attnv halving via block-diagonal v tiles (v_bd) + per-bank exp with
phase-reordered groups + psum re-plan.

v_bd per group: [128, 2048] bf16, per line li (cols 256*li..): 4 bank-tiles
[128, 64]: rows 0-63 = v[k(li), d of head b] in cols 0-31 (zeros in 32-63),
rows 64-127 = v[k(li), d of head b+4] in cols 32-63 (zeros in 0-31).
Built by 4 strided sbuf->sbuf DMAs per group from v_g; zero blocks are
memset once per pool buffer and never rewritten.

attnv per (hf, li, bw): ONE MM K=128, M=64, N=64:
  out rows [d_b | d_b+4] (b = 2hf+bw) = lhsT(v_bd)^T @ probs_bank_b.
"""

import numpy as np

B, T, C, H, W = 4, 16, 256, 64, 64
HEADS, D = 8, 32
NBT = B * T
NCORES = 8
BT_PER_CORE = NBT // NCORES  # 8
HW = H * W                   # 4096
L = W
GRP = 8
NGRP = H // GRP              # 8
GQ = GRP * L                 # 512

ST_COLS = 4356
VBD_BUFS = 8


def _build_bass():
    import concourse.bacc as bacc
    import concourse.mybir as mybir
    from concourse.tile import TileContext

    f32 = mybir.dt.float32
    f32r = mybir.dt.float32r
    bf16 = mybir.dt.bfloat16
    fp16 = mybir.dt.float16
    AF = mybir.ActivationFunctionType

    nc = bacc.Bacc("TRN2", target_bir_lowering=False, debug=False,
                   num_devices=NCORES)

    x_d = nc.dram_tensor("x", [BT_PER_CORE, C, HW], f32r, kind="ExternalInput").ap()
    st_d = nc.dram_tensor("statics", [128, ST_COLS], f32r, kind="ExternalInput").ap()
    out_d = nc.dram_tensor("out", [BT_PER_CORE, C, HW], f32, kind="ExternalOutput").ap()

    with TileContext(nc) as tc:
        with (
            tc.tile_pool(name="static", bufs=1) as stat,
            tc.tile_pool(name="xt", bufs=5) as pxt,
            tc.tile_pool(name="qk", bufs=44) as pqk,
            tc.tile_pool(name="vg", bufs=6) as pvg,
            tc.tile_pool(name="probs", bufs=4) as ppr,
            tc.tile_pool(name="rz", bufs=3) as prz,
            tc.tile_pool(name="osb", bufs=4) as po,
            tc.tile_pool(name="outsb", bufs=4) as pout,
            tc.tile_pool(name="psS", bufs=2, space="PSUM") as psS,
            tc.tile_pool(name="psZ", bufs=1, space="PSUM") as psZ,
            tc.tile_pool(name="psO", bufs=1, space="PSUM") as psO,
            tc.tile_pool(name="psP", bufs=1, space="PSUM") as psP,
        ):
            # ---- statics ----
            st = stat.tile([128, ST_COLS], f32r, tag="st", name="statics_sb")
            nc.sync.dma_start(out=st, in_=st_d)
            wqk = [st[:, 512 * i:512 * (i + 1)] for i in range(2)]
            wv = [st[:, 1024 + 256 * i:1024 + 256 * (i + 1)] for i in range(2)]
            wp = [[st[:, 1536 + 256 * i + 128 * j:1536 + 256 * i + 128 * (j + 1)]
                   for j in range(2)] for i in range(2)]
            expb_st = st[:, 2048:4096]
            bz_st = st[:, 4096:4352]
            bq = st[:, 4352:4354].bitcast(f32)
            bp = st[:, 4354:4356].bitcast(f32)
            expb_bf = stat.tile([128, 2048], bf16, tag="ebbf", name="expb_bf")
            bz_bf = stat.tile([128, 256], bf16, tag="bzbf", name="bz_bf")
            with nc.allow_low_precision(reason="exact 0/1 consts + bias"):
                nc.vector.tensor_copy(expb_bf, expb_st)
                nc.vector.tensor_copy(bz_bf, bz_st)

            # v_bd ring: raw sbuf tensors, zero blocks memset once and
            # never rewritten (Tile tracks WAR hazards by address).
            vbd_ring = [nc.alloc_sbuf_tensor(f"vbd{i}", [128, 2048], bf16).ap()
                        for i in range(VBD_BUFS)]
            for t in vbd_ring:
                nc.gpsimd.memset(t, 0.0)

            deferred = None
            for bt in range(BT_PER_CORE):
                # ---- load xT in [128, 2048] slices: xt[kc][xh] ----
                xt = [[pxt.tile([128, 2048], f32r, tag="xt", name="xt")
                       for _ in range(2)] for _ in range(2)]
                for kc in range(2):
                    for xh in range(2):
                        nc.sync.dma_start(
                            out=xt[kc][xh],
                            in_=x_d[bt, 128 * kc:128 * (kc + 1),
                                    2048 * xh:2048 * (xh + 1)])

                def xt_cols(kc, c0, w_):
                    xh = c0 // 2048
                    o = c0 - 2048 * xh
                    return xt[kc][xh][:, o:o + w_]

                # ---- interleaved qk + v projections ----
                # qk: per (mc, nn) -> qkt[(mc,nn)] bf16 [128,512]
                # v:  per group -> v_g -> v_sw (DMA) -> v_bd (gpsimd copies)
                qkt = {}
                v_bd = []

                def proj_ps(i, nm):
                    # alternate psum pools: psO idles during projections
                    if i % 2 == 0:
                        return psS.tile([128, 1024], f32, tag="ps", name=nm)
                    return psO.tile([128, 1024], f32, tag="o", name=nm)

                def qk_tile(np_, mc):
                    ps = proj_ps(mc, "psqk")
                    for j in range(2):
                        nn = 2 * np_ + j
                        for kc in range(2):
                            nc.tensor.matmul(
                                ps[:, 512 * j:512 * (j + 1)],
                                wqk[kc][:, 128 * mc:128 * (mc + 1)],
                                xt_cols(kc, 512 * nn, 512),
                                start=(kc == 0), stop=(kc == 1))
                    for j in range(2):
                        nn = 2 * np_ + j
                        dst = pqk.tile([128, 512], bf16, tag="qkT", name="qkT")
                        qkt[(mc, nn)] = dst
                        src = ps[:, 512 * j:512 * (j + 1)]
                        if mc < 2:   # q: fold b_q; j alternates engine
                            if j == 0:
                                nc.scalar.activation(
                                    dst, src, AF.Identity,
                                    bias=bq[:, mc:mc + 1], scale=1.0)
                            else:
                                with nc.allow_low_precision(reason="bias add"):
                                    nc.vector.tensor_scalar_add(
                                        dst, src, bq[:, mc:mc + 1])
                        else:        # k: plain copy; j alternates engine
                            if j == 0:
                                nc.scalar.copy(dst, src)
                            else:
                                nc.vector.tensor_copy(dst, src)

                def v_tile(g):
                    ps = proj_ps(g, "psv")
                    for pl in range(4):
                        pc = 4 * g + pl
                        for kc in range(2):
                            nc.tensor.matmul(
                                ps[:, 256 * pl:256 * (pl + 1)],
                                xt_cols(kc, 128 * pc, 128),
                                wv[kc], start=(kc == 0), stop=(kc == 1))
                    vg = pvg.tile([128, 1024], bf16, tag="v", name="v")
                    nc.scalar.copy(vg[:, 0:512], ps[:, 0:512])
                    nc.vector.tensor_copy(vg[:, 512:1024], ps[:, 512:1024])
                    vs = pvg.tile([128, 1024], bf16, tag="vsw", name="vsw")
                    nc.sync.dma_start(out=vs[0:64, :], in_=vg[64:128, :])
                    nc.sync.dma_start(out=vs[64:128, :], in_=vg[0:64, :])
                    vb = vbd_ring[(bt * NGRP + g) % VBD_BUFS]
                    # 4 partition-aligned strided copies on idle GPSIMD:
                    # (tb, p2): dst row-half tb, line parity p2; source vg
                    # when tb==p2 else the swapped copy vs.
                    for tb in range(2):
                        dst = vb[64 * tb:64 * (tb + 1), :].rearrange(
                            "p (pc l b j) -> p pc l b j", pc=4, l=2, b=4, j=64)
                        for p2 in range(2):
                            srcT = vg if tb == p2 else vs
                            src = srcT[64 * tb:64 * (tb + 1), :].rearrange(
                                "p (c hb j) -> p c hb j", c=4, hb=8, j=32)
                            with nc.allow_low_precision(reason="bf16 copy"):
                                nc.gpsimd.tensor_copy(
                                    dst[:, :, p2, :, 32 * tb:32 * (tb + 1)],
                                    src[:, :, 4 * tb:4 * (tb + 1), :])
                    v_bd.append(vb)

                for np_ in range(4):
                    for mc in range(4):
                        qk_tile(np_, mc)
                    v_tile(2 * np_)
                    v_tile(2 * np_ + 1)

                # ---- attention ----
                def flush_out(dfr):
                    dbt, dg, pps = dfr
                    for mc in range(2):
                        osb = pout.tile([128, 512], f32, tag="out", name="outsb")
                        if mc == 0:
                            nc.scalar.activation(
                                osb, pps[mc], AF.Identity,
                                bias=bp[:, mc:mc + 1], scale=1.0)
                        else:
                            with nc.allow_low_precision(reason="f32 add"):
                                nc.vector.tensor_scalar_add(
                                    osb, pps[mc], bp[:, mc:mc + 1])
                        nc.sync.dma_start(
                            out=out_d[dbt, 128 * mc:128 * (mc + 1),
                                      GQ * dg:GQ * (dg + 1)],
                            in_=osb)

                for g in range(NGRP):
                    if deferred is not None:
                        flush_out(deferred)
                        deferred = None
                    sps, probs = [], []
                    for hf in range(2):
                        sp = psS.tile([128, 1024], f32, tag="ps", name="psatt")
                        prr = ppr.tile([128, 1024], bf16, tag="praw", name="praw")
                        pr = ppr.tile([128, 1024], bf16, tag="probs", name="probs")
                        sps.append(sp)
                        probs.append(pr)
                        for bw in range(2):
                            for li in range(GRP):
                                for h in (2 * hf + bw, 2 * hf + 4 + bw):
                                    hc, hr = h // 4, h % 4
                                    kt = qkt[(2 + hc, g)][32 * hr:32 * (hr + 1),
                                                          64 * li:64 * (li + 1)]
                                    qt = qkt[(hc, g)][32 * hr:32 * (hr + 1),
                                                      64 * li:64 * (li + 1)]
                                    nc.tensor.matmul(
                                        sp[64 * hc:64 * (hc + 1),
                                           512 * bw + 64 * li:
                                           512 * bw + 64 * (li + 1)],
                                        kt, qt, start=True, stop=True,
                                        tile_position=(32 * hr, 64 * hc))
                            nc.scalar.activation(
                                prr[:, 512 * bw:512 * (bw + 1)],
                                sp[:, 512 * bw:512 * (bw + 1)], AF.Exp, scale=1.0)
                            with nc.allow_low_precision(reason="bf16 probs"):
                                nc.vector.tensor_mul(
                                    pr[:, 512 * bw:512 * (bw + 1)],
                                    prr[:, 512 * bw:512 * (bw + 1)],
                                    expb_bf[:, 1024 * hf + 512 * bw:
                                            1024 * hf + 512 * (bw + 1)])

                    ops = psO.tile([128, 1024], f32, tag="o", name="pso")
                    oT = []
                    for hf in range(2):
                        # attnv first: only needs probs
                        for bw in range(2):
                            for li in range(GRP):
                                b = 2 * hf + bw
                                vt = v_bd[g][:, 256 * li + 64 * b:
                                             256 * li + 64 * (b + 1)]
                                pt = probs[hf][:, 512 * bw + 64 * li:
                                               512 * bw + 64 * (li + 1)]
                                nc.tensor.matmul(
                                    ops[64 * bw:64 * (bw + 1),
                                        512 * hf + 64 * li:512 * hf + 64 * (li + 1)],
                                    vt, pt, start=True, stop=True,
                                    tile_position=(0, 64 * bw))
                        zps = psZ.tile([128, 512], f32, tag="z", name="psz")
                        for bw in range(2):
                            nc.tensor.matmul(
                                zps, bz_bf[:, 128 * bw:128 * (bw + 1)],
                                probs[hf][:, 512 * bw:512 * (bw + 1)],
                                start=(bw == 0), stop=(bw == 1))
                        rz = prz.tile([128, 512], f32, tag="rz", name="rz")
                        nc.vector.reciprocal_approx_fast(out=rz, in_=zps)
                        o = po.tile([128, 512], f32r, tag="oT", name="oT")
                        with nc.allow_low_precision(reason="f32r bits are f32"):
                            nc.vector.tensor_mul(
                                o, ops[:, 512 * hf:512 * (hf + 1)], rz)
                        oT.append(o)

                    pps = []
                    for mc in range(2):
                        if mc == 0:
                            pp = psP.tile([128, 512], f32, tag="proj", name="pspr")
                        else:
                            pp = psZ.tile([128, 512], f32, tag="z", name="pspr2")
                        for hf in range(2):
                            nc.tensor.matmul(pp, wp[hf][mc], oT[hf],
                                             start=(hf == 0), stop=(hf == 1))
                        pps.append(pp)
                    deferred = (bt, g, pps)
            if deferred is not None:
                flush_out(deferred)
    nc.compile()
    return nc


def _host_inputs(x, relative_bias, w_qkv, b_qkv, w_proj, b_proj):
    scale = D ** -0.5
    wq = w_qkv[:, :C] * scale
    wk = w_qkv[:, C:2 * C]
    wvm = w_qkv[:, 2 * C:]
    bqv = b_qkv[:C] * scale
    bv = b_qkv[2 * C:]
    wqk_full = np.concatenate([wq, wk], axis=1)
    perm = []
    for c_ in range(2):
        for h in (2 * c_, 2 * c_ + 4, 2 * c_ + 1, 2 * c_ + 5):
            perm.extend(range(32 * h, 32 * (h + 1)))
    wp_perm = w_proj[perm, :]
    # expb [128, 2048]: per (hf, bw): bank b = 2hf+bw, head h = b+4hh:
    # expb[64hh+k, 1024hf+512bw+64li+q] = exp(bias[h][q, k]), replicated
    # over the 8 lines of a group.
    expb = np.zeros((128, 2048), np.float32)
    for hf in range(2):
        for bw in range(2):
            b = 2 * hf + bw
            for hh in range(2):
                blk = np.exp(relative_bias[b + 4 * hh].T)      # [k, q]
                expb[64 * hh:64 * (hh + 1),
                     1024 * hf + 512 * bw:1024 * hf + 512 * (bw + 1)] = \
                    np.tile(blk, (1, GRP))
    bz = np.zeros((128, 256), np.float32)
    bz[0:64, 0:32] = 1.0
    bz[64:128, 32:64] = 1.0
    bz[0:64, 128 + 64:128 + 96] = 1.0
    bz[64:128, 128 + 96:128 + 128] = 1.0
    bq = np.stack([bqv[:128], bqv[128:]], axis=1).astype(np.float32)
    bpv = bv @ w_proj + b_proj
    bp = np.stack([bpv[:128], bpv[128:]], axis=1).astype(np.float32)
    st = np.zeros((128, ST_COLS), np.float32)
    st[:, 0:512] = wqk_full[:128]
    st[:, 512:1024] = wqk_full[128:]
    st[:, 1024:1280] = wvm[:128]
    st[:, 1280:1536] = wvm[128:]
    for kc in range(2):
        for mc in range(2):
            st[:, 1536 + 256 * kc + 128 * mc:1536 + 256 * kc + 128 * (mc + 1)] = \
                wp_perm[128 * kc:128 * (kc + 1), 128 * mc:128 * (mc + 1)]
    st[:, 2048:4096] = expb
    st[:, 4096:4352] = bz
    st[:, 4352:4354] = bq
    st[:, 4354:4356] = bp
    return dict(statics=st)


LAST_RESULTS = None


def kernel(x, relative_bias, w_qkv, b_qkv, w_proj, b_proj):
    import os
    import sys
    if '/opt/trn_rl_repo' not in sys.path:
        sys.path.insert(0, '/opt/trn_rl_repo')
    from concourse.bass_utils import run_bass_kernel_spmd

    x = np.asarray(x, np.float32)
    const = _host_inputs(x,
                         np.asarray(relative_bias, np.float32),
                         np.asarray(w_qkv, np.float32),
                         np.asarray(b_qkv, np.float32),
                         np.asarray(w_proj, np.float32),
                         np.asarray(b_proj, np.float32))
    xr = np.ascontiguousarray(x.reshape(NBT, C, HW))
    nc = _build_bass()
    in_maps = []
    for c in range(NCORES):
        m = dict(const)
        m["x"] = np.ascontiguousarray(xr[c * BT_PER_CORE:(c + 1) * BT_PER_CORE])
        in_maps.append(m)
    res = run_bass_kernel_spmd(nc, in_maps, list(range(NCORES)),
                               tmpdir=os.environ.get("BASS_TMPDIR"))
    global LAST_RESULTS
    LAST_RESULTS = res
    outs = res.results
    out = np.concatenate([o["out"].reshape(BT_PER_CORE, C, HW) for o in outs],
                         axis=0)
    return out.reshape(B, T, C, H, W).astype(np.float32)


# revision 5
# speedup vs baseline: 1.2655x; 1.0055x over previous
"""AxialAttention TRN2 Bass kernel — v2.

v1 + attnv halving via block-diagonal v tiles (v_bd) + per-bank exp with
phase-reordered groups + psum re-plan.

v_bd per group: [128, 2048] bf16, per line li (cols 256*li..): 4 bank-tiles
[128, 64]: rows 0-63 = v[k(li), d of head b] in cols 0-31 (zeros in 32-63),
rows 64-127 = v[k(li), d of head b+4] in cols 32-63 (zeros in 0-31).
Built by 4 strided sbuf->sbuf DMAs per group from v_g; zero blocks are
memset once per pool buffer and never rewritten.

attnv per (hf, li, bw): ONE MM K=128, M=64, N=64:
  out rows [d_b | d_b+4] (b = 2hf+bw) = lhsT(v_bd)^T @ probs_bank_b.
"""

import numpy as np

B, T, C, H, W = 4, 16, 256, 64, 64
HEADS, D = 8, 32
NBT = B * T
NCORES = 8
BT_PER_CORE = NBT // NCORES  # 8
HW = H * W                   # 4096
L = W
GRP = 8
NGRP = H // GRP              # 8
GQ = GRP * L                 # 512

ST_COLS = 4356
VBD_BUFS = 8


def _build_bass():
    import concourse.bacc as bacc
    import concourse.mybir as mybir
    from concourse.tile import TileContext

    f32 = mybir.dt.float32
    f32r = mybir.dt.float32r
    bf16 = mybir.dt.bfloat16
    fp16 = mybir.dt.float16
    AF = mybir.ActivationFunctionType

    nc = bacc.Bacc("TRN2", target_bir_lowering=False, debug=False,
                   num_devices=NCORES)

    x_d = nc.dram_tensor("x", [BT_PER_CORE, C, HW], f32r, kind="ExternalInput").ap()
    st_d = nc.dram_tensor("statics", [128, ST_COLS], f32r, kind="ExternalInput").ap()
    out_d = nc.dram_tensor("out", [BT_PER_CORE, C, HW], f32, kind="ExternalOutput").ap()

    with TileContext(nc) as tc:
        with (
            tc.tile_pool(name="static", bufs=1) as stat,
            tc.tile_pool(name="xt", bufs=5) as pxt,
            tc.tile_pool(name="qk", bufs=44) as pqk,
            tc.tile_pool(name="vg", bufs=6) as pvg,
            tc.tile_pool(name="probs", bufs=4) as ppr,
            tc.tile_pool(name="rz", bufs=3) as prz,
            tc.tile_pool(name="osb", bufs=4) as po,
            tc.tile_pool(name="outsb", bufs=4) as pout,
            tc.tile_pool(name="psS", bufs=4, space="PSUM") as psS,
            tc.tile_pool(name="psZ", bufs=1, space="PSUM") as psZ,
            tc.tile_pool(name="psO", bufs=1, space="PSUM") as psO,
            tc.tile_pool(name="psP", bufs=1, space="PSUM") as psP,
        ):
            # ---- statics ----
            st = stat.tile([128, ST_COLS], f32r, tag="st", name="statics_sb")
            nc.sync.dma_start(out=st[:, 0:1024], in_=st_d[:, 0:1024])
            nc.sync.dma_start(out=st[:, 1024:2048], in_=st_d[:, 1024:2048])
            nc.sync.dma_start(out=st[:, 4352:4356], in_=st_d[:, 4352:4356])
            nc.sync.dma_start(out=st[:, 2048:4352], in_=st_d[:, 2048:4352])
            wqk = [st[:, 512 * i:512 * (i + 1)] for i in range(2)]
            wv = [st[:, 1024 + 256 * i:1024 + 256 * (i + 1)] for i in range(2)]
            wp = [[st[:, 1536 + 256 * i + 128 * j:1536 + 256 * i + 128 * (j + 1)]
                   for j in range(2)] for i in range(2)]
            expb_st = st[:, 2048:4096]
            bz_st = st[:, 4096:4352]
            bq = st[:, 4352:4354].bitcast(f32)
            bp = st[:, 4354:4356].bitcast(f32)
            expb_bf = stat.tile([128, 2048], bf16, tag="ebbf", name="expb_bf")
            bz_bf = stat.tile([128, 256], bf16, tag="bzbf", name="bz_bf")
            with nc.allow_low_precision(reason="exact 0/1 consts + bias"):
                nc.vector.tensor_copy(expb_bf, expb_st)
                nc.vector.tensor_copy(bz_bf, bz_st)

            # v_bd ring: raw sbuf tensors, zero blocks memset once and
            # never rewritten (Tile tracks WAR hazards by address).
            vbd_ring = [nc.alloc_sbuf_tensor(f"vbd{i}", [128, 2048], bf16).ap()
                        for i in range(VBD_BUFS)]
            for t in vbd_ring:
                nc.gpsimd.memset(t, 0.0)

            deferred = None
            for bt in range(BT_PER_CORE):
                # ---- load xT in [128, 2048] slices: xt[kc][xh] ----
                xt = [[pxt.tile([128, 2048], f32r, tag="xt", name="xt")
                       for _ in range(2)] for _ in range(2)]
                for xh in range(2):
                    for kc in range(2):
                        nc.sync.dma_start(
                            out=xt[kc][xh],
                            in_=x_d[bt, 128 * kc:128 * (kc + 1),
                                    2048 * xh:2048 * (xh + 1)])

                def xt_cols(kc, c0, w_):
                    xh = c0 // 2048
                    o = c0 - 2048 * xh
                    return xt[kc][xh][:, o:o + w_]

                # ---- interleaved qk + v projections ----
                # qk: per (mc, nn) -> qkt[(mc,nn)] bf16 [128,512]
                # v:  per group -> v_g -> v_sw (DMA) -> v_bd (gpsimd copies)
                qkt = {}
                v_bd = []

                def qk_tile(g, mc):
                    # psum: mc0/1 from the sc pool; mc2 -> psZ, mc3 -> psP
                    # (those pools idle at this point of the group cycle).
                    if mc < 2:
                        ps = psS.tile([128, 512], f32, tag="sc", name="psqk")
                    elif mc == 2:
                        ps = psZ.tile([128, 512], f32, tag="z", name="psqk")
                    else:
                        ps = psP.tile([128, 512], f32, tag="proj", name="psqk")
                    for kc in range(2):
                        nc.tensor.matmul(
                            ps,
                            wqk[kc][:, 128 * mc:128 * (mc + 1)],
                            xt_cols(kc, 512 * g, 512),
                            start=(kc == 0), stop=(kc == 1))
                    dst = pqk.tile([128, 512], bf16, tag="qkT", name="qkT")
                    qkt[(mc, g)] = dst
                    if mc < 2:   # q: fold b_q; mc alternates engine
                        if mc == 0:
                            nc.scalar.activation(
                                dst, ps, AF.Identity,
                                bias=bq[:, mc:mc + 1], scale=1.0)
                        else:
                            with nc.allow_low_precision(reason="bias add"):
                                nc.vector.tensor_scalar_add(
                                    dst, ps, bq[:, mc:mc + 1])
                    else:        # k: plain copy; mc alternates engine
                        if mc == 2:
                            nc.scalar.copy(dst, ps)
                        else:
                            nc.vector.tensor_copy(dst, ps)

                def v_tile(g):
                    ph = (psS.tile([128, 512], f32, tag="sc", name="psv"),
                          psS.tile([128, 512], f32, tag="sc", name="psv"))
                    for pl in range(4):
                        pc = 4 * g + pl
                        for kc in range(2):
                            nc.tensor.matmul(
                                ph[pl // 2][:, 256 * (pl % 2):256 * (pl % 2 + 1)],
                                xt_cols(kc, 128 * pc, 128),
                                wv[kc], start=(kc == 0), stop=(kc == 1))
                    vg = pvg.tile([128, 1024], bf16, tag="v", name="v")
                    nc.scalar.copy(vg[:, 0:512], ph[0])
                    nc.vector.tensor_copy(vg[:, 512:1024], ph[1])
                    vs = pvg.tile([128, 1024], bf16, tag="vsw", name="vsw")
                    nc.sync.dma_start(out=vs[0:64, :], in_=vg[64:128, :])
                    nc.sync.dma_start(out=vs[64:128, :], in_=vg[0:64, :])
                    vb = vbd_ring[(bt * NGRP + g) % VBD_BUFS]
                    # 4 partition-aligned strided copies on idle GPSIMD:
                    # (tb, p2): dst row-half tb, line parity p2; source vg
                    # when tb==p2 else the swapped copy vs.
                    for tb in range(2):
                        dst = vb[64 * tb:64 * (tb + 1), :].rearrange(
                            "p (pc l b j) -> p pc l b j", pc=4, l=2, b=4, j=64)
                        for p2 in range(2):
                            srcT = vg if tb == p2 else vs
                            src = srcT[64 * tb:64 * (tb + 1), :].rearrange(
                                "p (c hb j) -> p c hb j", c=4, hb=8, j=32)
                            with nc.allow_low_precision(reason="bf16 copy"):
                                nc.gpsimd.tensor_copy(
                                    dst[:, :, p2, :, 32 * tb:32 * (tb + 1)],
                                    src[:, :, 4 * tb:4 * (tb + 1), :])
                    v_bd.append(vb)

                # ---- per-group: projections then attention ----
                def flush_out(dfr):
                    dbt, dg, pps = dfr
                    for mc in range(2):
                        osb = pout.tile([128, 512], f32, tag="out", name="outsb")
                        if mc == 0:
                            nc.scalar.activation(
                                osb, pps[mc], AF.Identity,
                                bias=bp[:, mc:mc + 1], scale=1.0)
                        else:
                            with nc.allow_low_precision(reason="f32 add"):
                                nc.vector.tensor_scalar_add(
                                    osb, pps[mc], bp[:, mc:mc + 1])
                        nc.sync.dma_start(
                            out=out_d[dbt, 128 * mc:128 * (mc + 1),
                                      GQ * dg:GQ * (dg + 1)],
                            in_=osb)

                for gi in range(NGRP + 2):
                    g = gi - 2
                    if g < 0:
                        for mc in range(4):
                            qk_tile(gi, mc)
                        v_tile(gi)
                        continue
                    if deferred is not None:
                        flush_out(deferred)
                        deferred = None
                    probs = []
                    for hf in range(2):
                        prr = ppr.tile([128, 1024], bf16, tag="praw", name="praw")
                        pr = ppr.tile([128, 1024], bf16, tag="probs", name="probs")
                        probs.append(pr)
                        for bw in range(2):
                            sp = psS.tile([128, 512], f32, tag="sc", name="psatt")
                            for li in range(GRP):
                                for h in (2 * hf + bw, 2 * hf + 4 + bw):
                                    hc, hr = h // 4, h % 4
                                    kt = qkt[(2 + hc, g)][32 * hr:32 * (hr + 1),
                                                          64 * li:64 * (li + 1)]
                                    qt = qkt[(hc, g)][32 * hr:32 * (hr + 1),
                                                      64 * li:64 * (li + 1)]
                                    nc.tensor.matmul(
                                        sp[64 * hc:64 * (hc + 1),
                                           64 * li:64 * (li + 1)],
                                        kt, qt, start=True, stop=True,
                                        tile_position=(32 * hr, 64 * hc))
                            nc.scalar.activation(
                                prr[:, 512 * bw:512 * (bw + 1)],
                                sp, AF.Exp, scale=1.0)
                            with nc.allow_low_precision(reason="bf16 probs"):
                                nc.vector.tensor_mul(
                                    pr[:, 512 * bw:512 * (bw + 1)],
                                    prr[:, 512 * bw:512 * (bw + 1)],
                                    expb_bf[:, 1024 * hf + 512 * bw:
                                            1024 * hf + 512 * (bw + 1)])

                    if gi < NGRP:
                        for mc in range(4):
                            qk_tile(gi, mc)
                        v_tile(gi)

                    ops = psO.tile([128, 1024], f32, tag="o", name="pso")
                    oT = []
                    for hf in range(2):
                        # attnv first: only needs probs
                        for bw in range(2):
                            for li in range(GRP):
                                b = 2 * hf + bw
                                vt = v_bd[g][:, 256 * li + 64 * b:
                                             256 * li + 64 * (b + 1)]
                                pt = probs[hf][:, 512 * bw + 64 * li:
                                               512 * bw + 64 * (li + 1)]
                                nc.tensor.matmul(
                                    ops[64 * bw:64 * (bw + 1),
                                        512 * hf + 64 * li:512 * hf + 64 * (li + 1)],
                                    vt, pt, start=True, stop=True,
                                    tile_position=(0, 64 * bw))
                        zps = psZ.tile([128, 512], f32, tag="z", name="psz")
                        # (psZ also hosts qk-mc2 and proj-mc1 tiles)
                        for bw in range(2):
                            nc.tensor.matmul(
                                zps, bz_bf[:, 128 * bw:128 * (bw + 1)],
                                probs[hf][:, 512 * bw:512 * (bw + 1)],
                                start=(bw == 0), stop=(bw == 1))
                        rz = prz.tile([128, 512], f32, tag="rz", name="rz")
                        nc.vector.reciprocal_approx_fast(out=rz, in_=zps)
                        o = po.tile([128, 512], f32r, tag="oT", name="oT")
                        with nc.allow_low_precision(reason="f32r bits are f32"):
                            nc.vector.tensor_mul(
                                o, ops[:, 512 * hf:512 * (hf + 1)], rz)
                        oT.append(o)

                    pps = []
                    for mc in range(2):
                        if mc == 0:
                            pp = psP.tile([128, 512], f32, tag="proj", name="pspr")
                        else:
                            pp = psZ.tile([128, 512], f32, tag="z", name="pspr2")
                        for hf in range(2):
                            nc.tensor.matmul(pp, wp[hf][mc], oT[hf],
                                             start=(hf == 0), stop=(hf == 1))
                        pps.append(pp)
                    deferred = (bt, g, pps)
            if deferred is not None:
                flush_out(deferred)
    nc.compile()
    return nc


def _host_inputs(x, relative_bias, w_qkv, b_qkv, w_proj, b_proj):
    scale = D ** -0.5
    wq = w_qkv[:, :C] * scale
    wk = w_qkv[:, C:2 * C]
    wvm = w_qkv[:, 2 * C:]
    bqv = b_qkv[:C] * scale
    bv = b_qkv[2 * C:]
    wqk_full = np.concatenate([wq, wk], axis=1)
    perm = []
    for c_ in range(2):
        for h in (2 * c_, 2 * c_ + 4, 2 * c_ + 1, 2 * c_ + 5):
            perm.extend(range(32 * h, 32 * (h + 1)))
    wp_perm = w_proj[perm, :]
    # expb [128, 2048]: per (hf, bw): bank b = 2hf+bw, head h = b+4hh:
    # expb[64hh+k, 1024hf+512bw+64li+q] = exp(bias[h][q, k]), replicated
    # over the 8 lines of a group.
    expb = np.zeros((128, 2048), np.float32)
    for hf in range(2):
        for bw in range(2):
            b = 2 * hf + bw
            for hh in range(2):
                blk = np.exp(relative_bias[b + 4 * hh].T)      # [k, q]
                expb[64 * hh:64 * (hh + 1),
                     1024 * hf + 512 * bw:1024 * hf + 512 * (bw + 1)] = \
                    np.tile(blk, (1, GRP))
    bz = np.zeros((128, 256), np.float32)
    bz[0:64, 0:32] = 1.0
    bz[64:128, 32:64] = 1.0
    bz[0:64, 128 + 64:128 + 96] = 1.0
    bz[64:128, 128 + 96:128 + 128] = 1.0
    bq = np.stack([bqv[:128], bqv[128:]], axis=1).astype(np.float32)
    bpv = bv @ w_proj + b_proj
    bp = np.stack([bpv[:128], bpv[128:]], axis=1).astype(np.float32)
    st = np.zeros((128, ST_COLS), np.float32)
    st[:, 0:512] = wqk_full[:128]
    st[:, 512:1024] = wqk_full[128:]
    st[:, 1024:1280] = wvm[:128]
    st[:, 1280:1536] = wvm[128:]
    for kc in range(2):
        for mc in range(2):
            st[:, 1536 + 256 * kc + 128 * mc:1536 + 256 * kc + 128 * (mc + 1)] = \
                wp_perm[128 * kc:128 * (kc + 1), 128 * mc:128 * (mc + 1)]
    st[:, 2048:4096] = expb
    st[:, 4096:4352] = bz
    st[:, 4352:4354] = bq
    st[:, 4354:4356] = bp
    return dict(statics=st)


LAST_RESULTS = None


def kernel(x, relative_bias, w_qkv, b_qkv, w_proj, b_proj):
    import os
    import sys
    if '/opt/trn_rl_repo' not in sys.path:
        sys.path.insert(0, '/opt/trn_rl_repo')
    from concourse.bass_utils import run_bass_kernel_spmd

    x = np.asarray(x, np.float32)
    const = _host_inputs(x,
                         np.asarray(relative_bias, np.float32),
                         np.asarray(w_qkv, np.float32),
                         np.asarray(b_qkv, np.float32),
                         np.asarray(w_proj, np.float32),
                         np.asarray(b_proj, np.float32))
    xr = np.ascontiguousarray(x.reshape(NBT, C, HW))
    nc = _build_bass()
    in_maps = []
    for c in range(NCORES):
        m = dict(const)
        m["x"] = np.ascontiguousarray(xr[c * BT_PER_CORE:(c + 1) * BT_PER_CORE])
        in_maps.append(m)
    res = run_bass_kernel_spmd(nc, in_maps, list(range(NCORES)),
                               tmpdir=os.environ.get("BASS_TMPDIR"))
    global LAST_RESULTS
    LAST_RESULTS = res
    outs = res.results
    out = np.concatenate([o["out"].reshape(BT_PER_CORE, C, HW) for o in outs],
                         axis=0)
    return out.reshape(B, T, C, H, W).astype(np.float32)


# revision 6
# speedup vs baseline: 1.2674x; 1.0015x over previous
"""AxialAttention TRN2 Bass kernel.

Shapes (hardcoded): x [B=4,T=16,C=256,H=64,W=64] fp32.
N = B*T*H = 4096 lines of [L=64, C=256]; heads=8, d=32.
Sharding: 64 (b,t) blocks -> 8 per core across 8 cores (data parallel).

Per-core dataflow — software-pipelined at group granularity (a group is
8 lines = 512 positions); projections for group gi run 2 groups ahead of
attention for group gi-2, interleaved in one loop so PE stays dense and
ACT/DVE evacuation load is spread evenly:

  qkT  = w_qk^T @ xT      fp32r MMs N=512, q pre-scaled by 1/sqrt(d);
                          b_q folded into the psum->sbuf evacuation.
  v    = xT^T @ w_v       row-major v; v_sw = partition-swapped copy (DMA);
                          v_bd = per-line block-diagonal tiles built by 4
                          strided GPSIMD copies into a pre-zeroed ring of
                          raw sbuf tensors ([d_b | d_b+4] columns, zeros
                          in the cross blocks).
  per group g, half hf (banks b = 2hf+bw, heads {2hf+bw, 2hf+4+bw}):
    scoresT[k,q] = kT_h^T @ qT_h      bf16, tile-position packed, one
                                      [128,512] psum bank per (hf,bw)
    praw  = exp(scoresT)              ACT, per bank
    probs = praw * exp(bias)          DVE bf16 (replaces psum bias seeds)
    Zbc   = Bz^T @ probs              2 MMs -> broadcast Z [128,512]
                                      (0/1 Bz matrices; no compact-Z /
                                      broadcast stage needed)
    rzbc  = 1/Zbc                     DVE reciprocal_approx_fast
    oT    = v_bd^T @ probs            16 MMs K=128 M=64 N=64, rows
                                      [h, h+4, h+1, h+5] per chunk
    oT_n  = oT * rzbc                 DVE
    proj += wp[hf]^T @ oT_n           fp32r, accumulated over halves
  out = proj + (b_v @ w_proj + b_proj)   evacuated one group deferred
                                         (ACT mc0 / DVE mc1), then DMA.

PSUM (16KB/partition, exactly full): 4x [128,512] "sc" (scores + q/k
projection chunks), 1x [128,1024] "o" (attnv out pair), 1x [128,512] "z"
(Zbc + k-proj mc2 + out-proj mc1), 1x [128,512] "proj" (k-proj mc3 +
out-proj mc0).
"""

import numpy as np

B, T, C, H, W = 4, 16, 256, 64, 64
HEADS, D = 8, 32
NBT = B * T
NCORES = 8
BT_PER_CORE = NBT // NCORES  # 8
HW = H * W                   # 4096
L = W
GRP = 8
NGRP = H // GRP              # 8
GQ = GRP * L                 # 512

ST_COLS = 4356
VBD_BUFS = 8


def _build_bass():
    import concourse.bacc as bacc
    import concourse.mybir as mybir
    from concourse.tile import TileContext

    f32 = mybir.dt.float32
    f32r = mybir.dt.float32r
    bf16 = mybir.dt.bfloat16
    fp16 = mybir.dt.float16
    AF = mybir.ActivationFunctionType

    nc = bacc.Bacc("TRN2", target_bir_lowering=False, debug=False,
                   num_devices=NCORES)

    x_d = nc.dram_tensor("x", [BT_PER_CORE, C, HW], f32r, kind="ExternalInput").ap()
    st_d = nc.dram_tensor("statics", [128, ST_COLS], f32r, kind="ExternalInput").ap()
    out_d = nc.dram_tensor("out", [BT_PER_CORE, C, HW], f32, kind="ExternalOutput").ap()

    with TileContext(nc) as tc:
        with (
            tc.tile_pool(name="static", bufs=1) as stat,
            tc.tile_pool(name="xt", bufs=5) as pxt,
            tc.tile_pool(name="qk", bufs=24) as pqk,
            tc.tile_pool(name="vg", bufs=6) as pvg,
            tc.tile_pool(name="probs", bufs=6) as ppr,
            tc.tile_pool(name="rz", bufs=3) as prz,
            tc.tile_pool(name="osb", bufs=4) as po,
            tc.tile_pool(name="outsb", bufs=4) as pout,
            tc.tile_pool(name="psS", bufs=4, space="PSUM") as psS,
            tc.tile_pool(name="psZ", bufs=1, space="PSUM") as psZ,
            tc.tile_pool(name="psO", bufs=1, space="PSUM") as psO,
            tc.tile_pool(name="psP", bufs=1, space="PSUM") as psP,
        ):
            # ---- statics ----
            st = stat.tile([128, ST_COLS], f32r, tag="st", name="statics_sb")
            nc.sync.dma_start(out=st[:, 0:1024], in_=st_d[:, 0:1024])
            nc.sync.dma_start(out=st[:, 1024:2048], in_=st_d[:, 1024:2048])
            nc.sync.dma_start(out=st[:, 4352:4356], in_=st_d[:, 4352:4356])
            nc.sync.dma_start(out=st[:, 2048:4352], in_=st_d[:, 2048:4352])
            wqk = [st[:, 512 * i:512 * (i + 1)] for i in range(2)]
            wv = [st[:, 1024 + 256 * i:1024 + 256 * (i + 1)] for i in range(2)]
            wp = [[st[:, 1536 + 256 * i + 128 * j:1536 + 256 * i + 128 * (j + 1)]
                   for j in range(2)] for i in range(2)]
            expb_st = st[:, 2048:4096]
            bz_st = st[:, 4096:4352]
            bq = st[:, 4352:4354].bitcast(f32)
            bp = st[:, 4354:4356].bitcast(f32)
            expb_bf = stat.tile([128, 2048], bf16, tag="ebbf", name="expb_bf")
            bz_bf = stat.tile([128, 256], bf16, tag="bzbf", name="bz_bf")
            with nc.allow_low_precision(reason="exact 0/1 consts + bias"):
                nc.vector.tensor_copy(expb_bf, expb_st)
                nc.vector.tensor_copy(bz_bf, bz_st)

            # v_bd ring: raw sbuf tensors, zero blocks memset once and
            # never rewritten (Tile tracks WAR hazards by address).
            vbd_ring = [nc.alloc_sbuf_tensor(f"vbd{i}", [128, 2048], bf16).ap()
                        for i in range(VBD_BUFS)]
            for t in vbd_ring:
                nc.gpsimd.memset(t, 0.0)

            deferred = None
            for bt in range(BT_PER_CORE):
                # ---- load xT in [128, 2048] slices: xt[kc][xh] ----
                xt = [[pxt.tile([128, 2048], f32r, tag="xt", name="xt")
                       for _ in range(2)] for _ in range(2)]
                for xh in range(2):
                    for kc in range(2):
                        nc.sync.dma_start(
                            out=xt[kc][xh],
                            in_=x_d[bt, 128 * kc:128 * (kc + 1),
                                    2048 * xh:2048 * (xh + 1)])

                def xt_cols(kc, c0, w_):
                    xh = c0 // 2048
                    o = c0 - 2048 * xh
                    return xt[kc][xh][:, o:o + w_]

                # ---- interleaved qk + v projections ----
                # qk: per (mc, nn) -> qkt[(mc,nn)] bf16 [128,512]
                # v:  per group -> v_g -> v_sw (DMA) -> v_bd (gpsimd copies)
                qkt = {}
                v_bd = []

                def qk_tile(g, mc):
                    # psum: mc0/1 from the sc pool; mc2 -> psZ, mc3 -> psP
                    # (those pools idle at this point of the group cycle).
                    if mc < 2:
                        ps = psS.tile([128, 512], f32, tag="sc", name="psqk")
                    elif mc == 2:
                        ps = psZ.tile([128, 512], f32, tag="z", name="psqk")
                    else:
                        ps = psP.tile([128, 512], f32, tag="proj", name="psqk")
                    for kc in range(2):
                        nc.tensor.matmul(
                            ps,
                            wqk[kc][:, 128 * mc:128 * (mc + 1)],
                            xt_cols(kc, 512 * g, 512),
                            start=(kc == 0), stop=(kc == 1))
                    dst = pqk.tile([128, 512], bf16, tag="qkT", name="qkT")
                    qkt[(mc, g)] = dst
                    if mc < 2:   # q: fold b_q; mc alternates engine
                        if mc == 0:
                            nc.scalar.activation(
                                dst, ps, AF.Identity,
                                bias=bq[:, mc:mc + 1], scale=1.0)
                        else:
                            with nc.allow_low_precision(reason="bias add"):
                                nc.vector.tensor_scalar_add(
                                    dst, ps, bq[:, mc:mc + 1])
                    else:        # k: plain copy; mc alternates engine
                        if mc == 2:
                            nc.scalar.copy(dst, ps)
                        else:
                            nc.vector.tensor_copy(dst, ps)

                def v_tile(g):
                    ph = (psS.tile([128, 512], f32, tag="sc", name="psv"),
                          psS.tile([128, 512], f32, tag="sc", name="psv"))
                    for pl in range(4):
                        pc = 4 * g + pl
                        for kc in range(2):
                            nc.tensor.matmul(
                                ph[pl // 2][:, 256 * (pl % 2):256 * (pl % 2 + 1)],
                                xt_cols(kc, 128 * pc, 128),
                                wv[kc], start=(kc == 0), stop=(kc == 1))
                    vg = pvg.tile([128, 1024], bf16, tag="v", name="v")
                    nc.scalar.copy(vg[:, 0:512], ph[0])
                    nc.vector.tensor_copy(vg[:, 512:1024], ph[1])
                    vs = pvg.tile([128, 1024], bf16, tag="vsw", name="vsw")
                    nc.sync.dma_start(out=vs[0:64, :], in_=vg[64:128, :])
                    nc.sync.dma_start(out=vs[64:128, :], in_=vg[0:64, :])
                    vb = vbd_ring[(bt * NGRP + g) % VBD_BUFS]
                    # 4 partition-aligned strided copies on idle GPSIMD:
                    # (tb, p2): dst row-half tb, line parity p2; source vg
                    # when tb==p2 else the swapped copy vs.
                    for tb in range(2):
                        dst = vb[64 * tb:64 * (tb + 1), :].rearrange(
                            "p (pc l b j) -> p pc l b j", pc=4, l=2, b=4, j=64)
                        for p2 in range(2):
                            srcT = vg if tb == p2 else vs
                            src = srcT[64 * tb:64 * (tb + 1), :].rearrange(
                                "p (c hb j) -> p c hb j", c=4, hb=8, j=32)
                            with nc.allow_low_precision(reason="bf16 copy"):
                                nc.gpsimd.tensor_copy(
                                    dst[:, :, p2, :, 32 * tb:32 * (tb + 1)],
                                    src[:, :, 4 * tb:4 * (tb + 1), :])
                    v_bd.append(vb)

                # ---- per-group: projections then attention ----
                def flush_out(dfr):
                    dbt, dg, pps = dfr
                    for mc in range(2):
                        osb = pout.tile([128, 512], f32, tag="out", name="outsb")
                        if mc == 0:
                            nc.scalar.activation(
                                osb, pps[mc], AF.Identity,
                                bias=bp[:, mc:mc + 1], scale=1.0)
                        else:
                            with nc.allow_low_precision(reason="f32 add"):
                                nc.vector.tensor_scalar_add(
                                    osb, pps[mc], bp[:, mc:mc + 1])
                        nc.sync.dma_start(
                            out=out_d[dbt, 128 * mc:128 * (mc + 1),
                                      GQ * dg:GQ * (dg + 1)],
                            in_=osb)

                for gi in range(NGRP + 2):
                    g = gi - 2
                    if g < 0:
                        for mc in range(4):
                            qk_tile(gi, mc)
                        v_tile(gi)
                        continue
                    if deferred is not None:
                        flush_out(deferred)
                        deferred = None
                    probs = []
                    for hf in range(2):
                        prr = ppr.tile([128, 1024], bf16, tag="praw", name="praw")
                        pr = ppr.tile([128, 1024], bf16, tag="probs", name="probs")
                        probs.append(pr)
                        for bw in range(2):
                            sp = psS.tile([128, 512], f32, tag="sc", name="psatt")
                            for li in range(GRP):
                                for h in (2 * hf + bw, 2 * hf + 4 + bw):
                                    hc, hr = h // 4, h % 4
                                    kt = qkt[(2 + hc, g)][32 * hr:32 * (hr + 1),
                                                          64 * li:64 * (li + 1)]
                                    qt = qkt[(hc, g)][32 * hr:32 * (hr + 1),
                                                      64 * li:64 * (li + 1)]
                                    nc.tensor.matmul(
                                        sp[64 * hc:64 * (hc + 1),
                                           64 * li:64 * (li + 1)],
                                        kt, qt, start=True, stop=True,
                                        tile_position=(32 * hr, 64 * hc))
                            nc.scalar.activation(
                                prr[:, 512 * bw:512 * (bw + 1)],
                                sp, AF.Exp, scale=1.0)
                            with nc.allow_low_precision(reason="bf16 probs"):
                                nc.vector.tensor_mul(
                                    pr[:, 512 * bw:512 * (bw + 1)],
                                    prr[:, 512 * bw:512 * (bw + 1)],
                                    expb_bf[:, 1024 * hf + 512 * bw:
                                            1024 * hf + 512 * (bw + 1)])

                    if gi < NGRP:
                        for mc in range(4):
                            qk_tile(gi, mc)
                        v_tile(gi)

                    ops = psO.tile([128, 1024], f32, tag="o", name="pso")
                    oT = []
                    for hf in range(2):
                        # attnv first: only needs probs
                        for bw in range(2):
                            for li in range(GRP):
                                b = 2 * hf + bw
                                vt = v_bd[g][:, 256 * li + 64 * b:
                                             256 * li + 64 * (b + 1)]
                                pt = probs[hf][:, 512 * bw + 64 * li:
                                               512 * bw + 64 * (li + 1)]
                                nc.tensor.matmul(
                                    ops[64 * bw:64 * (bw + 1),
                                        512 * hf + 64 * li:512 * hf + 64 * (li + 1)],
                                    vt, pt, start=True, stop=True,
                                    tile_position=(0, 64 * bw))
                        zps = psZ.tile([128, 512], f32, tag="z", name="psz")
                        # (psZ also hosts qk-mc2 and proj-mc1 tiles)
                        for bw in range(2):
                            nc.tensor.matmul(
                                zps, bz_bf[:, 128 * bw:128 * (bw + 1)],
                                probs[hf][:, 512 * bw:512 * (bw + 1)],
                                start=(bw == 0), stop=(bw == 1))
                        rz = prz.tile([128, 512], f32, tag="rz", name="rz")
                        nc.vector.reciprocal_approx_fast(out=rz, in_=zps)
                        o = po.tile([128, 512], f32r, tag="oT", name="oT")
                        with nc.allow_low_precision(reason="f32r bits are f32"):
                            nc.vector.tensor_mul(
                                o, ops[:, 512 * hf:512 * (hf + 1)], rz)
                        oT.append(o)

                    pps = []
                    for mc in range(2):
                        if mc == 0:
                            pp = psP.tile([128, 512], f32, tag="proj", name="pspr")
                        else:
                            pp = psZ.tile([128, 512], f32, tag="z", name="pspr2")
                        for hf in range(2):
                            nc.tensor.matmul(pp, wp[hf][mc], oT[hf],
                                             start=(hf == 0), stop=(hf == 1))
                        pps.append(pp)
                    deferred = (bt, g, pps)
            if deferred is not None:
                flush_out(deferred)
    nc.compile()
    return nc


def _host_inputs(x, relative_bias, w_qkv, b_qkv, w_proj, b_proj):
    scale = D ** -0.5
    wq = w_qkv[:, :C] * scale
    wk = w_qkv[:, C:2 * C]
    wvm = w_qkv[:, 2 * C:]
    bqv = b_qkv[:C] * scale
    bv = b_qkv[2 * C:]
    wqk_full = np.concatenate([wq, wk], axis=1)
    perm = []
    for c_ in range(2):
        for h in (2 * c_, 2 * c_ + 4, 2 * c_ + 1, 2 * c_ + 5):
            perm.extend(range(32 * h, 32 * (h + 1)))
    wp_perm = w_proj[perm, :]
    # expb [128, 2048]: per (hf, bw): bank b = 2hf+bw, head h = b+4hh:
    # expb[64hh+k, 1024hf+512bw+64li+q] = exp(bias[h][q, k]), replicated
    # over the 8 lines of a group.
    expb = np.zeros((128, 2048), np.float32)
    for hf in range(2):
        for bw in range(2):
            b = 2 * hf + bw
            for hh in range(2):
                blk = np.exp(relative_bias[b + 4 * hh].T)      # [k, q]
                expb[64 * hh:64 * (hh + 1),
                     1024 * hf + 512 * bw:1024 * hf + 512 * (bw + 1)] = \
                    np.tile(blk, (1, GRP))
    bz = np.zeros((128, 256), np.float32)
    bz[0:64, 0:32] = 1.0
    bz[64:128, 32:64] = 1.0
    bz[0:64, 128 + 64:128 + 96] = 1.0
    bz[64:128, 128 + 96:128 + 128] = 1.0
    bq = np.stack([bqv[:128], bqv[128:]], axis=1).astype(np.float32)
    bpv = bv @ w_proj + b_proj
    bp = np.stack([bpv[:128], bpv[128:]], axis=1).astype(np.float32)
    st = np.zeros((128, ST_COLS), np.float32)
    st[:, 0:512] = wqk_full[:128]
    st[:, 512:1024] = wqk_full[128:]
    st[:, 1024:1280] = wvm[:128]
    st[:, 1280:1536] = wvm[128:]
    for kc in range(2):
        for mc in range(2):
            st[:, 1536 + 256 * kc + 128 * mc:1536 + 256 * kc + 128 * (mc + 1)] = \
                wp_perm[128 * kc:128 * (kc + 1), 128 * mc:128 * (mc + 1)]
    st[:, 2048:4096] = expb
    st[:, 4096:4352] = bz
    st[:, 4352:4354] = bq
    st[:, 4354:4356] = bp
    return dict(statics=st)


LAST_RESULTS = None


def kernel(x, relative_bias, w_qkv, b_qkv, w_proj, b_proj):
    import os
    import sys
    if '/opt/trn_rl_repo' not in sys.path:
        sys.path.insert(0, '/opt/trn_rl_repo')
    from concourse.bass_utils import run_bass_kernel_spmd

    x = np.asarray(x, np.float32)
    const = _host_inputs(x,
                         np.asarray(relative_bias, np.float32),
                         np.asarray(w_qkv, np.float32),
                         np.asarray(b_qkv, np.float32),
                         np.asarray(w_proj, np.float32),
                         np.asarray(b_proj, np.float32))
    xr = np.ascontiguousarray(x.reshape(NBT, C, HW))
    nc = _build_bass()
    in_maps = []
    for c in range(NCORES):
        m = dict(const)
        m["x"] = np.ascontiguousarray(xr[c * BT_PER_CORE:(c + 1) * BT_PER_CORE])
        in_maps.append(m)
    res = run_bass_kernel_spmd(nc, in_maps, list(range(NCORES)),
                               tmpdir=os.environ.get("BASS_TMPDIR"))
    global LAST_RESULTS
    LAST_RESULTS = res
    outs = res.results
    out = np.concatenate([o["out"].reshape(BT_PER_CORE, C, HW) for o in outs],
                         axis=0)
    return out.reshape(B, T, C, H, W).astype(np.float32)


# revision 7
# speedup vs baseline: 1.2734x; 1.0047x over previous
"""AxialAttention TRN2 Bass kernel.

Shapes (hardcoded): x [B=4,T=16,C=256,H=64,W=64] fp32.
N = B*T*H = 4096 lines of [L=64, C=256]; heads=8, d=32.
Sharding: 64 (b,t) blocks -> 8 per core across 8 cores (data parallel).

Per-core dataflow — software-pipelined at group granularity (a group is
8 lines = 512 positions); projections for group gi run 2 groups ahead of
attention for group gi-2, interleaved in one loop so PE stays dense and
ACT/DVE evacuation load is spread evenly:

  qkT  = w_qk^T @ xT      fp32r MMs N=512, q pre-scaled by 1/sqrt(d);
                          b_q folded into the psum->sbuf evacuation.
  v    = xT^T @ w_v       row-major v; v_sw = partition-swapped copy (DMA);
                          v_bd = per-line block-diagonal tiles built by 4
                          strided GPSIMD copies into a pre-zeroed ring of
                          raw sbuf tensors ([d_b | d_b+4] columns, zeros
                          in the cross blocks).
  per group g, half hf (banks b = 2hf+bw, heads {2hf+bw, 2hf+4+bw}):
    scoresT[k,q] = kT_h^T @ qT_h      bf16, tile-position packed, one
                                      [128,512] psum bank per (hf,bw)
    praw  = exp(scoresT)              ACT, per bank
    probs = praw * exp(bias)          DVE bf16 (replaces psum bias seeds)
    Zbc   = Bz^T @ probs              2 MMs -> broadcast Z [128,512]
                                      (0/1 Bz matrices; no compact-Z /
                                      broadcast stage needed)
    rzbc  = 1/Zbc                     DVE reciprocal_approx_fast
    oT    = v_bd^T @ probs            16 MMs K=128 M=64 N=64, rows
                                      [h, h+4, h+1, h+5] per chunk
    oT_n  = oT * rzbc                 DVE
    proj += wp[hf]^T @ oT_n           fp32r, accumulated over halves
  out = proj + (b_v @ w_proj + b_proj)   evacuated one group deferred
                                         (ACT mc0 / DVE mc1), then DMA.

PSUM (16KB/partition, exactly full): 4x [128,512] "sc" (scores + q/k
projection chunks), 1x [128,1024] "o" (attnv out pair), 1x [128,512] "z"
(Zbc + k-proj mc2 + out-proj mc1), 1x [128,512] "proj" (k-proj mc3 +
out-proj mc0).
"""

import numpy as np

B, T, C, H, W = 4, 16, 256, 64, 64
HEADS, D = 8, 32
NBT = B * T
NCORES = 8
BT_PER_CORE = NBT // NCORES  # 8
HW = H * W                   # 4096
L = W
GRP = 8
NGRP = H // GRP              # 8
GQ = GRP * L                 # 512

ST_COLS = 4356
VBD_BUFS = 8


def _build_bass():
    import concourse.bacc as bacc
    import concourse.mybir as mybir
    from concourse.tile import TileContext

    f32 = mybir.dt.float32
    f32r = mybir.dt.float32r
    bf16 = mybir.dt.bfloat16
    fp16 = mybir.dt.float16
    AF = mybir.ActivationFunctionType

    nc = bacc.Bacc("TRN2", target_bir_lowering=False, debug=False,
                   num_devices=NCORES)

    x_d = nc.dram_tensor("x", [BT_PER_CORE, C, HW], f32r, kind="ExternalInput").ap()
    st_d = nc.dram_tensor("statics", [128, ST_COLS], f32r, kind="ExternalInput").ap()
    out_d = nc.dram_tensor("out", [BT_PER_CORE, C, HW], f32, kind="ExternalOutput").ap()

    with TileContext(nc) as tc:
        with (
            tc.tile_pool(name="static", bufs=1) as stat,
            tc.tile_pool(name="xt", bufs=5) as pxt,
            tc.tile_pool(name="qk", bufs=24) as pqk,
            tc.tile_pool(name="vg", bufs=6) as pvg,
            tc.tile_pool(name="probs", bufs=6) as ppr,
            tc.tile_pool(name="rz", bufs=3) as prz,
            tc.tile_pool(name="osb", bufs=4) as po,
            tc.tile_pool(name="outsb", bufs=4) as pout,
            tc.tile_pool(name="psS", bufs=4, space="PSUM") as psS,
            tc.tile_pool(name="psZ", bufs=1, space="PSUM") as psZ,
            tc.tile_pool(name="psO", bufs=2, space="PSUM") as psO,
            tc.tile_pool(name="psP", bufs=1, space="PSUM") as psP,
        ):
            # ---- statics ----
            st = stat.tile([128, ST_COLS], f32r, tag="st", name="statics_sb")
            nc.sync.dma_start(out=st[:, 0:1024], in_=st_d[:, 0:1024])
            nc.sync.dma_start(out=st[:, 1024:2048], in_=st_d[:, 1024:2048])
            nc.sync.dma_start(out=st[:, 4352:4356], in_=st_d[:, 4352:4356])
            nc.sync.dma_start(out=st[:, 2048:4352], in_=st_d[:, 2048:4352])
            wqk = [st[:, 512 * i:512 * (i + 1)] for i in range(2)]
            wv = [st[:, 1024 + 256 * i:1024 + 256 * (i + 1)] for i in range(2)]
            wp = [[st[:, 1536 + 256 * i + 128 * j:1536 + 256 * i + 128 * (j + 1)]
                   for j in range(2)] for i in range(2)]
            expb_st = st[:, 2048:4096]
            bz_st = st[:, 4096:4352]
            bq = st[:, 4352:4354].bitcast(f32)
            bp = st[:, 4354:4356].bitcast(f32)
            expb_bf = stat.tile([128, 2048], bf16, tag="ebbf", name="expb_bf")
            bz_bf = stat.tile([128, 256], bf16, tag="bzbf", name="bz_bf")
            with nc.allow_low_precision(reason="exact 0/1 consts + bias"):
                nc.vector.tensor_copy(expb_bf, expb_st)
                nc.vector.tensor_copy(bz_bf, bz_st)

            # v_bd ring: raw sbuf tensors, zero blocks memset once and
            # never rewritten (Tile tracks WAR hazards by address).
            vbd_ring = [nc.alloc_sbuf_tensor(f"vbd{i}", [128, 2048], bf16).ap()
                        for i in range(VBD_BUFS)]
            for t in vbd_ring:
                nc.gpsimd.memset(t, 0.0)

            deferred = None
            for bt in range(BT_PER_CORE):
                # ---- load xT in [128, 2048] slices: xt[kc][xh] ----
                xt = [[pxt.tile([128, 2048], f32r, tag="xt", name="xt")
                       for _ in range(2)] for _ in range(2)]
                for xh in range(2):
                    for kc in range(2):
                        nc.sync.dma_start(
                            out=xt[kc][xh],
                            in_=x_d[bt, 128 * kc:128 * (kc + 1),
                                    2048 * xh:2048 * (xh + 1)])

                def xt_cols(kc, c0, w_):
                    xh = c0 // 2048
                    o = c0 - 2048 * xh
                    return xt[kc][xh][:, o:o + w_]

                # ---- interleaved qk + v projections ----
                # qk: per (mc, nn) -> qkt[(mc,nn)] bf16 [128,512]
                # v:  per group -> v_g -> v_sw (DMA) -> v_bd (gpsimd copies)
                qkt = {}
                v_bd = []

                def qk_tile(g, mc):
                    # psum: mc0/1 from the sc pool; mc2 -> psZ, mc3 -> psP
                    # (those pools idle at this point of the group cycle).
                    if mc < 2:
                        ps = psS.tile([128, 512], f32, tag="sc", name="psqk")
                    elif mc == 2:
                        ps = psZ.tile([128, 512], f32, tag="z", name="psqk")
                    else:
                        ps = psP.tile([128, 512], f32, tag="proj", name="psqk")
                    for kc in range(2):
                        nc.tensor.matmul(
                            ps,
                            wqk[kc][:, 128 * mc:128 * (mc + 1)],
                            xt_cols(kc, 512 * g, 512),
                            start=(kc == 0), stop=(kc == 1))
                    dst = pqk.tile([128, 512], bf16, tag="qkT", name="qkT")
                    qkt[(mc, g)] = dst
                    if mc < 2:   # q: fold b_q; mc alternates engine
                        if mc == 0:
                            nc.scalar.activation(
                                dst, ps, AF.Identity,
                                bias=bq[:, mc:mc + 1], scale=1.0)
                        else:
                            with nc.allow_low_precision(reason="bias add"):
                                nc.vector.tensor_scalar_add(
                                    dst, ps, bq[:, mc:mc + 1])
                    else:        # k: plain copy; mc alternates engine
                        if mc == 2:
                            nc.scalar.copy(dst, ps)
                        else:
                            nc.vector.tensor_copy(dst, ps)

                def v_tile(g):
                    ph = (psS.tile([128, 512], f32, tag="sc", name="psv"),
                          psS.tile([128, 512], f32, tag="sc", name="psv"))
                    for pl in range(4):
                        pc = 4 * g + pl
                        for kc in range(2):
                            nc.tensor.matmul(
                                ph[pl // 2][:, 256 * (pl % 2):256 * (pl % 2 + 1)],
                                xt_cols(kc, 128 * pc, 128),
                                wv[kc], start=(kc == 0), stop=(kc == 1))
                    vg = pvg.tile([128, 1024], bf16, tag="v", name="v")
                    nc.scalar.copy(vg[:, 0:512], ph[0])
                    nc.vector.tensor_copy(vg[:, 512:1024], ph[1])
                    vs = pvg.tile([128, 1024], bf16, tag="vsw", name="vsw")
                    nc.sync.dma_start(out=vs[0:64, :], in_=vg[64:128, :])
                    nc.sync.dma_start(out=vs[64:128, :], in_=vg[0:64, :])
                    vb = vbd_ring[(bt * NGRP + g) % VBD_BUFS]
                    # 4 partition-aligned strided copies on idle GPSIMD:
                    # (tb, p2): dst row-half tb, line parity p2; source vg
                    # when tb==p2 else the swapped copy vs.
                    for tb in range(2):
                        dst = vb[64 * tb:64 * (tb + 1), :].rearrange(
                            "p (pc l b j) -> p pc l b j", pc=4, l=2, b=4, j=64)
                        for p2 in range(2):
                            srcT = vg if tb == p2 else vs
                            src = srcT[64 * tb:64 * (tb + 1), :].rearrange(
                                "p (c hb j) -> p c hb j", c=4, hb=8, j=32)
                            with nc.allow_low_precision(reason="bf16 copy"):
                                nc.gpsimd.tensor_copy(
                                    dst[:, :, p2, :, 32 * tb:32 * (tb + 1)],
                                    src[:, :, 4 * tb:4 * (tb + 1), :])
                    v_bd.append(vb)

                # ---- per-group: projections then attention ----
                def flush_out(dfr):
                    dbt, dg, pps = dfr
                    for mc in range(2):
                        osb = pout.tile([128, 512], f32, tag="out", name="outsb")
                        if mc == 0:
                            nc.scalar.activation(
                                osb, pps[mc], AF.Identity,
                                bias=bp[:, mc:mc + 1], scale=1.0)
                        else:
                            with nc.allow_low_precision(reason="f32 add"):
                                nc.vector.tensor_scalar_add(
                                    osb, pps[mc], bp[:, mc:mc + 1])
                        nc.sync.dma_start(
                            out=out_d[dbt, 128 * mc:128 * (mc + 1),
                                      GQ * dg:GQ * (dg + 1)],
                            in_=osb)

                for gi in range(NGRP + 2):
                    g = gi - 2
                    if g < 0:
                        for mc in range(4):
                            qk_tile(gi, mc)
                        v_tile(gi)
                        continue
                    if deferred is not None:
                        flush_out(deferred)
                        deferred = None
                    probs = []
                    for hf in range(2):
                        prr = ppr.tile([128, 1024], bf16, tag="praw", name="praw")
                        pr = ppr.tile([128, 1024], bf16, tag="probs", name="probs")
                        probs.append(pr)
                        for bw in range(2):
                            sp = psS.tile([128, 512], f32, tag="sc", name="psatt")
                            for li in range(GRP):
                                for h in (2 * hf + bw, 2 * hf + 4 + bw):
                                    hc, hr = h // 4, h % 4
                                    kt = qkt[(2 + hc, g)][32 * hr:32 * (hr + 1),
                                                          64 * li:64 * (li + 1)]
                                    qt = qkt[(hc, g)][32 * hr:32 * (hr + 1),
                                                      64 * li:64 * (li + 1)]
                                    nc.tensor.matmul(
                                        sp[64 * hc:64 * (hc + 1),
                                           64 * li:64 * (li + 1)],
                                        kt, qt, start=True, stop=True,
                                        tile_position=(32 * hr, 64 * hc))
                            nc.scalar.activation(
                                prr[:, 512 * bw:512 * (bw + 1)],
                                sp, AF.Exp, scale=1.0)
                            with nc.allow_low_precision(reason="bf16 probs"):
                                nc.vector.tensor_mul(
                                    pr[:, 512 * bw:512 * (bw + 1)],
                                    prr[:, 512 * bw:512 * (bw + 1)],
                                    expb_bf[:, 1024 * hf + 512 * bw:
                                            1024 * hf + 512 * (bw + 1)])

                    if gi < NGRP:
                        for mc in range(4):
                            qk_tile(gi, mc)
                        v_tile(gi)

                    oT = []
                    for hf in range(2):
                        ops = psO.tile([128, 512], f32, tag="o", name="pso")
                        # attnv first: only needs probs
                        for bw in range(2):
                            for li in range(GRP):
                                b = 2 * hf + bw
                                vt = v_bd[g][:, 256 * li + 64 * b:
                                             256 * li + 64 * (b + 1)]
                                pt = probs[hf][:, 512 * bw + 64 * li:
                                               512 * bw + 64 * (li + 1)]
                                nc.tensor.matmul(
                                    ops[64 * bw:64 * (bw + 1),
                                        64 * li:64 * (li + 1)],
                                    vt, pt, start=True, stop=True,
                                    tile_position=(0, 64 * bw))
                        zps = psZ.tile([128, 512], f32, tag="z", name="psz")
                        # (psZ also hosts qk-mc2 and proj-mc1 tiles)
                        for bw in range(2):
                            nc.tensor.matmul(
                                zps, bz_bf[:, 128 * bw:128 * (bw + 1)],
                                probs[hf][:, 512 * bw:512 * (bw + 1)],
                                start=(bw == 0), stop=(bw == 1))
                        rz = prz.tile([128, 512], f32, tag="rz", name="rz")
                        nc.vector.reciprocal_approx_fast(out=rz, in_=zps)
                        o = po.tile([128, 512], f32r, tag="oT", name="oT")
                        with nc.allow_low_precision(reason="f32r bits are f32"):
                            nc.vector.tensor_mul(o, ops, rz)
                        oT.append(o)

                    pps = []
                    for mc in range(2):
                        if mc == 0:
                            pp = psP.tile([128, 512], f32, tag="proj", name="pspr")
                        else:
                            pp = psZ.tile([128, 512], f32, tag="z", name="pspr2")
                        for hf in range(2):
                            nc.tensor.matmul(pp, wp[hf][mc], oT[hf],
                                             start=(hf == 0), stop=(hf == 1))
                        pps.append(pp)
                    deferred = (bt, g, pps)
            if deferred is not None:
                flush_out(deferred)
    nc.compile()
    return nc


def _host_inputs(x, relative_bias, w_qkv, b_qkv, w_proj, b_proj):
    scale = D ** -0.5
    wq = w_qkv[:, :C] * scale
    wk = w_qkv[:, C:2 * C]
    wvm = w_qkv[:, 2 * C:]
    bqv = b_qkv[:C] * scale
    bv = b_qkv[2 * C:]
    wqk_full = np.concatenate([wq, wk], axis=1)
    perm = []
    for c_ in range(2):
        for h in (2 * c_, 2 * c_ + 4, 2 * c_ + 1, 2 * c_ + 5):
            perm.extend(range(32 * h, 32 * (h + 1)))
    wp_perm = w_proj[perm, :]
    # expb [128, 2048]: per (hf, bw): bank b = 2hf+bw, head h = b+4hh:
    # expb[64hh+k, 1024hf+512bw+64li+q] = exp(bias[h][q, k]), replicated
    # over the 8 lines of a group.
    expb = np.zeros((128, 2048), np.float32)
    for hf in range(2):
        for bw in range(2):
            b = 2 * hf + bw
            for hh in range(2):
                blk = np.exp(relative_bias[b + 4 * hh].T)      # [k, q]
                expb[64 * hh:64 * (hh + 1),
                     1024 * hf + 512 * bw:1024 * hf + 512 * (bw + 1)] = \
                    np.tile(blk, (1, GRP))
    bz = np.zeros((128, 256), np.float32)
    bz[0:64, 0:32] = 1.0
    bz[64:128, 32:64] = 1.0
    bz[0:64, 128 + 64:128 + 96] = 1.0
    bz[64:128, 128 + 96:128 + 128] = 1.0
    bq = np.stack([bqv[:128], bqv[128:]], axis=1).astype(np.float32)
    bpv = bv @ w_proj + b_proj
    bp = np.stack([bpv[:128], bpv[128:]], axis=1).astype(np.float32)
    st = np.zeros((128, ST_COLS), np.float32)
    st[:, 0:512] = wqk_full[:128]
    st[:, 512:1024] = wqk_full[128:]
    st[:, 1024:1280] = wvm[:128]
    st[:, 1280:1536] = wvm[128:]
    for kc in range(2):
        for mc in range(2):
            st[:, 1536 + 256 * kc + 128 * mc:1536 + 256 * kc + 128 * (mc + 1)] = \
                wp_perm[128 * kc:128 * (kc + 1), 128 * mc:128 * (mc + 1)]
    st[:, 2048:4096] = expb
    st[:, 4096:4352] = bz
    st[:, 4352:4354] = bq
    st[:, 4354:4356] = bp
    return dict(statics=st)


LAST_RESULTS = None


def kernel(x, relative_bias, w_qkv, b_qkv, w_proj, b_proj):
    import os
    import sys
    if '/opt/trn_rl_repo' not in sys.path:
        sys.path.insert(0, '/opt/trn_rl_repo')
    from concourse.bass_utils import run_bass_kernel_spmd

    x = np.asarray(x, np.float32)
    const = _host_inputs(x,
                         np.asarray(relative_bias, np.float32),
                         np.asarray(w_qkv, np.float32),
                         np.asarray(b_qkv, np.float32),
                         np.asarray(w_proj, np.float32),
                         np.asarray(b_proj, np.float32))
    xr = np.ascontiguousarray(x.reshape(NBT, C, HW))
    nc = _build_bass()
    in_maps = []
    for c in range(NCORES):
        m = dict(const)
        m["x"] = np.ascontiguousarray(xr[c * BT_PER_CORE:(c + 1) * BT_PER_CORE])
        in_maps.append(m)
    res = run_bass_kernel_spmd(nc, in_maps, list(range(NCORES)),
                               tmpdir=os.environ.get("BASS_TMPDIR"))
    global LAST_RESULTS
    LAST_RESULTS = res
    outs = res.results
    out = np.concatenate([o["out"].reshape(BT_PER_CORE, C, HW) for o in outs],
                         axis=0)
    return out.reshape(B, T, C, H, W).astype(np.float32)


# revision 8
# speedup vs baseline: 1.2784x; 1.0040x over previous
"""AxialAttention TRN2 Bass kernel.

Shapes (hardcoded): x [B=4,T=16,C=256,H=64,W=64] fp32.
N = B*T*H = 4096 lines of [L=64, C=256]; heads=8, d=32.
Sharding: 64 (b,t) blocks -> 8 per core across 8 cores (data parallel).

Per-core dataflow — software-pipelined at group granularity (a group is
8 lines = 512 positions); projections for group gi run 2 groups ahead of
attention for group gi-2, interleaved in one loop so PE stays dense and
ACT/DVE evacuation load is spread evenly:

  qkT  = w_qk^T @ xT      fp32r MMs N=512, q pre-scaled by 1/sqrt(d);
                          b_q folded into the psum->sbuf evacuation.
  v    = xT^T @ w_v       row-major v; v_sw = partition-swapped copy (DMA);
                          v_bd = per-line block-diagonal tiles built by 4
                          strided GPSIMD copies into a pre-zeroed ring of
                          raw sbuf tensors ([d_b | d_b+4] columns, zeros
                          in the cross blocks).
  per group g, half hf (banks b = 2hf+bw, heads {2hf+bw, 2hf+4+bw}):
    scoresT[k,q] = kT_h^T @ qT_h      bf16, tile-position packed, one
                                      [128,512] psum bank per (hf,bw)
    praw  = exp(scoresT)              ACT, per bank
    probs = praw * exp(bias)          DVE bf16 (replaces psum bias seeds)
    Zbc   = Bz^T @ probs              2 MMs -> broadcast Z [128,512]
                                      (0/1 Bz matrices; no compact-Z /
                                      broadcast stage needed)
    rzbc  = 1/Zbc                     DVE reciprocal_approx_fast
    oT    = v_bd^T @ probs            16 MMs K=128 M=64 N=64, rows
                                      [h, h+4, h+1, h+5] per chunk
    oT_n  = oT * rzbc                 DVE
    proj += wp[hf]^T @ oT_n           fp32r, accumulated over halves
  out = proj + (b_v @ w_proj + b_proj)   evacuated one group deferred
                                         (ACT mc0 / DVE mc1), then DMA.

PSUM (16KB/partition, exactly full): 4x [128,512] "sc" (scores + q/k
projection chunks), 1x [128,1024] "o" (attnv out pair), 1x [128,512] "z"
(Zbc + k-proj mc2 + out-proj mc1), 1x [128,512] "proj" (k-proj mc3 +
out-proj mc0).
"""

import numpy as np

B, T, C, H, W = 4, 16, 256, 64, 64
HEADS, D = 8, 32
NBT = B * T
NCORES = 8
BT_PER_CORE = NBT // NCORES  # 8
HW = H * W                   # 4096
L = W
GRP = 8
NGRP = H // GRP              # 8
GQ = GRP * L                 # 512

ST_COLS = 4356
VBD_BUFS = 8


def _build_bass():
    import concourse.bacc as bacc
    import concourse.mybir as mybir
    from concourse.tile import TileContext

    f32 = mybir.dt.float32
    f32r = mybir.dt.float32r
    bf16 = mybir.dt.bfloat16
    fp16 = mybir.dt.float16
    AF = mybir.ActivationFunctionType

    nc = bacc.Bacc("TRN2", target_bir_lowering=False, debug=False,
                   num_devices=NCORES)

    x_d = nc.dram_tensor("x", [BT_PER_CORE, C, HW], f32r, kind="ExternalInput").ap()
    st_d = nc.dram_tensor("statics", [128, ST_COLS], f32r, kind="ExternalInput").ap()
    out_d = nc.dram_tensor("out", [BT_PER_CORE, C, HW], f32, kind="ExternalOutput").ap()

    with TileContext(nc) as tc:
        with (
            tc.tile_pool(name="static", bufs=1) as stat,
            tc.tile_pool(name="xt", bufs=5) as pxt,
            tc.tile_pool(name="qk", bufs=24) as pqk,
            tc.tile_pool(name="vg", bufs=6) as pvg,
            tc.tile_pool(name="probs", bufs=6) as ppr,
            tc.tile_pool(name="rz", bufs=3) as prz,
            tc.tile_pool(name="osb", bufs=4) as po,
            tc.tile_pool(name="outsb", bufs=4) as pout,
            tc.tile_pool(name="psS", bufs=4, space="PSUM") as psS,
            tc.tile_pool(name="psZ", bufs=1, space="PSUM") as psZ,
            tc.tile_pool(name="psO", bufs=2, space="PSUM") as psO,
            tc.tile_pool(name="psP", bufs=1, space="PSUM") as psP,
        ):
            # ---- statics ----
            st = stat.tile([128, ST_COLS], f32r, tag="st", name="statics_sb")
            nc.sync.dma_start(out=st[:, 0:1024], in_=st_d[:, 0:1024])
            nc.sync.dma_start(out=st[:, 1024:2048], in_=st_d[:, 1024:2048])
            nc.sync.dma_start(out=st[:, 4352:4356], in_=st_d[:, 4352:4356])
            wqk = [st[:, 512 * i:512 * (i + 1)] for i in range(2)]
            wv = [st[:, 1024 + 256 * i:1024 + 256 * (i + 1)] for i in range(2)]
            wp = [[st[:, 1536 + 256 * i + 128 * j:1536 + 256 * i + 128 * (j + 1)]
                   for j in range(2)] for i in range(2)]
            expb_st = st[:, 2048:4096]
            bz_st = st[:, 4096:4352]
            bq = st[:, 4352:4354].bitcast(f32)
            bp = st[:, 4354:4356].bitcast(f32)
            expb_bf = stat.tile([128, 2048], bf16, tag="ebbf", name="expb_bf")
            bz_bf = stat.tile([128, 256], bf16, tag="bzbf", name="bz_bf")

            # v_bd ring: raw sbuf tensors, zero blocks memset once and
            # never rewritten (Tile tracks WAR hazards by address).
            vbd_ring = [nc.alloc_sbuf_tensor(f"vbd{i}", [128, 2048], bf16).ap()
                        for i in range(VBD_BUFS)]
            for t in vbd_ring:
                nc.gpsimd.memset(t, 0.0)

            deferred = []
            for bt in range(BT_PER_CORE):
                # ---- load xT in [128, 2048] slices: xt[kc][xh] ----
                xt = [[pxt.tile([128, 2048], f32r, tag="xt", name="xt")
                       for _ in range(2)] for _ in range(2)]
                for xh in range(2):
                    for kc in range(2):
                        if bt == 0 and xh == 0:
                            nc.sync.dma_start(
                                out=xt[kc][xh][:, 0:512],
                                in_=x_d[bt, 128 * kc:128 * (kc + 1), 0:512])
                            nc.sync.dma_start(
                                out=xt[kc][xh][:, 512:2048],
                                in_=x_d[bt, 128 * kc:128 * (kc + 1),
                                        512:2048])
                        else:
                            nc.sync.dma_start(
                                out=xt[kc][xh],
                                in_=x_d[bt, 128 * kc:128 * (kc + 1),
                                        2048 * xh:2048 * (xh + 1)])

                if bt == 0:
                    # expb/bz arrive after block-0 inputs (needed ~12us in)
                    nc.sync.dma_start(out=st[:, 2048:4352],
                                      in_=st_d[:, 2048:4352])
                    with nc.allow_low_precision(reason="exact 0/1 + bias"):
                        nc.vector.tensor_copy(expb_bf, expb_st)
                        nc.vector.tensor_copy(bz_bf, bz_st)

                def xt_cols(kc, c0, w_):
                    xh = c0 // 2048
                    o = c0 - 2048 * xh
                    return xt[kc][xh][:, o:o + w_]

                # ---- interleaved qk + v projections ----
                # qk: per (mc, nn) -> qkt[(mc,nn)] bf16 [128,512]
                # v:  per group -> v_g -> v_sw (DMA) -> v_bd (gpsimd copies)
                qkt = {}
                v_bd = []

                def qk_tile(g, mc):
                    # psum: mc0/1 from the sc pool; mc2 -> psZ, mc3 -> psP
                    # (those pools idle at this point of the group cycle).
                    if mc < 2:
                        ps = psO.tile([128, 512], f32, tag="o", name="psqk")
                    elif mc == 2:
                        ps = psZ.tile([128, 512], f32, tag="z", name="psqk")
                    else:
                        ps = psP.tile([128, 512], f32, tag="proj", name="psqk")
                    for kc in range(2):
                        nc.tensor.matmul(
                            ps,
                            wqk[kc][:, 128 * mc:128 * (mc + 1)],
                            xt_cols(kc, 512 * g, 512),
                            start=(kc == 0), stop=(kc == 1))
                    dst = pqk.tile([128, 512], bf16, tag="qkT", name="qkT")
                    qkt[(mc, g)] = dst
                    if mc < 2:   # q: fold b_q; mc alternates engine
                        if mc == 0:
                            nc.scalar.activation(
                                dst, ps, AF.Identity,
                                bias=bq[:, mc:mc + 1], scale=1.0)
                        else:
                            with nc.allow_low_precision(reason="bias add"):
                                nc.vector.tensor_scalar_add(
                                    dst, ps, bq[:, mc:mc + 1])
                    else:        # k: plain copy; mc alternates engine
                        if mc == 2:
                            nc.scalar.copy(dst, ps)
                        else:
                            nc.vector.tensor_copy(dst, ps)

                def v_tile(g):
                    ph = (psS.tile([128, 512], f32, tag="sc", name="psv"),
                          psS.tile([128, 512], f32, tag="sc", name="psv"))
                    for pl in range(4):
                        pc = 4 * g + pl
                        for kc in range(2):
                            nc.tensor.matmul(
                                ph[pl // 2][:, 256 * (pl % 2):256 * (pl % 2 + 1)],
                                xt_cols(kc, 128 * pc, 128),
                                wv[kc], start=(kc == 0), stop=(kc == 1))
                    vg = pvg.tile([128, 1024], bf16, tag="v", name="v")
                    nc.scalar.copy(vg[:, 0:512], ph[0])
                    nc.vector.tensor_copy(vg[:, 512:1024], ph[1])
                    vs = pvg.tile([128, 1024], bf16, tag="vsw", name="vsw")
                    nc.sync.dma_start(out=vs[0:64, :], in_=vg[64:128, :])
                    nc.sync.dma_start(out=vs[64:128, :], in_=vg[0:64, :])
                    vb = vbd_ring[(bt * NGRP + g) % VBD_BUFS]
                    # 4 partition-aligned strided copies on idle GPSIMD:
                    # (tb, p2): dst row-half tb, line parity p2; source vg
                    # when tb==p2 else the swapped copy vs.
                    for tb in range(2):
                        dst = vb[64 * tb:64 * (tb + 1), :].rearrange(
                            "p (pc l b j) -> p pc l b j", pc=4, l=2, b=4, j=64)
                        for p2 in range(2):
                            srcT = vg if tb == p2 else vs
                            src = srcT[64 * tb:64 * (tb + 1), :].rearrange(
                                "p (c hb j) -> p c hb j", c=4, hb=8, j=32)
                            with nc.allow_low_precision(reason="bf16 copy"):
                                nc.gpsimd.tensor_copy(
                                    dst[:, :, p2, :, 32 * tb:32 * (tb + 1)],
                                    src[:, :, 4 * tb:4 * (tb + 1), :])
                    v_bd.append(vb)

                # ---- per-group: projections then attention ----
                def flush_out(dfr):
                    dbt, dg, pps = dfr
                    for mc in range(2):
                        osb = pout.tile([128, 512], f32, tag="out", name="outsb")
                        if mc == 0:
                            nc.scalar.activation(
                                osb, pps[mc], AF.Identity,
                                bias=bp[:, mc:mc + 1], scale=1.0)
                        else:
                            with nc.allow_low_precision(reason="f32 add"):
                                nc.vector.tensor_scalar_add(
                                    osb, pps[mc], bp[:, mc:mc + 1])
                        nc.sync.dma_start(
                            out=out_d[dbt, 128 * mc:128 * (mc + 1),
                                      GQ * dg:GQ * (dg + 1)],
                            in_=osb)

                for gi in range(NGRP + 2):
                    g = gi - 2
                    if g < 0:
                        for mc in range(4):
                            qk_tile(gi, mc)
                        v_tile(gi)
                        continue
                    probs = []
                    for hf in range(2):
                        prr = ppr.tile([128, 1024], bf16, tag="praw", name="praw")
                        pr = ppr.tile([128, 1024], bf16, tag="probs", name="probs")
                        probs.append(pr)
                        for bw in range(2):
                            sp = psS.tile([128, 512], f32, tag="sc", name="psatt")
                            for li in range(GRP):
                                for h in (2 * hf + bw, 2 * hf + 4 + bw):
                                    hc, hr = h // 4, h % 4
                                    kt = qkt[(2 + hc, g)][32 * hr:32 * (hr + 1),
                                                          64 * li:64 * (li + 1)]
                                    qt = qkt[(hc, g)][32 * hr:32 * (hr + 1),
                                                      64 * li:64 * (li + 1)]
                                    nc.tensor.matmul(
                                        sp[64 * hc:64 * (hc + 1),
                                           64 * li:64 * (li + 1)],
                                        kt, qt, start=True, stop=True,
                                        tile_position=(32 * hr, 64 * hc))
                            nc.scalar.activation(
                                prr[:, 512 * bw:512 * (bw + 1)],
                                sp, AF.Exp, scale=1.0)
                            with nc.allow_low_precision(reason="bf16 probs"):
                                nc.vector.tensor_mul(
                                    pr[:, 512 * bw:512 * (bw + 1)],
                                    prr[:, 512 * bw:512 * (bw + 1)],
                                    expb_bf[:, 1024 * hf + 512 * bw:
                                            1024 * hf + 512 * (bw + 1)])
                            if hf == 0 and bw == 0 and deferred:
                                flush_out(deferred.pop(0))

                    if gi < NGRP:
                        for mc in range(4):
                            qk_tile(gi, mc)
                        v_tile(gi)

                    oT = []
                    for hf in range(2):
                        ops = psO.tile([128, 512], f32, tag="o", name="pso")
                        # attnv first: only needs probs
                        for bw in range(2):
                            for li in range(GRP):
                                b = 2 * hf + bw
                                vt = v_bd[g][:, 256 * li + 64 * b:
                                             256 * li + 64 * (b + 1)]
                                pt = probs[hf][:, 512 * bw + 64 * li:
                                               512 * bw + 64 * (li + 1)]
                                nc.tensor.matmul(
                                    ops[64 * bw:64 * (bw + 1),
                                        64 * li:64 * (li + 1)],
                                    vt, pt, start=True, stop=True,
                                    tile_position=(0, 64 * bw))
                        zps = psZ.tile([128, 512], f32, tag="z", name="psz")
                        # (psZ also hosts qk-mc2 and proj-mc1 tiles)
                        for bw in range(2):
                            nc.tensor.matmul(
                                zps, bz_bf[:, 128 * bw:128 * (bw + 1)],
                                probs[hf][:, 512 * bw:512 * (bw + 1)],
                                start=(bw == 0), stop=(bw == 1))
                        rz = prz.tile([128, 512], f32, tag="rz", name="rz")
                        nc.vector.reciprocal_approx_fast(out=rz, in_=zps)
                        o = po.tile([128, 512], f32r, tag="oT", name="oT")
                        with nc.allow_low_precision(reason="f32r bits are f32"):
                            nc.vector.tensor_mul(o, ops, rz)
                        oT.append(o)

                    pps = []
                    for mc in range(2):
                        if mc == 0:
                            pp = psP.tile([128, 512], f32, tag="proj", name="pspr")
                        else:
                            pp = psZ.tile([128, 512], f32, tag="z", name="pspr2")
                        for hf in range(2):
                            nc.tensor.matmul(pp, wp[hf][mc], oT[hf],
                                             start=(hf == 0), stop=(hf == 1))
                        pps.append(pp)
                    deferred.append((bt, g, pps))
            for dfr in deferred:
                flush_out(dfr)
            deferred = []
    nc.compile()
    return nc


def _host_inputs(x, relative_bias, w_qkv, b_qkv, w_proj, b_proj):
    scale = D ** -0.5
    wq = w_qkv[:, :C] * scale
    wk = w_qkv[:, C:2 * C]
    wvm = w_qkv[:, 2 * C:]
    bqv = b_qkv[:C] * scale
    bv = b_qkv[2 * C:]
    wqk_full = np.concatenate([wq, wk], axis=1)
    perm = []
    for c_ in range(2):
        for h in (2 * c_, 2 * c_ + 4, 2 * c_ + 1, 2 * c_ + 5):
            perm.extend(range(32 * h, 32 * (h + 1)))
    wp_perm = w_proj[perm, :]
    # expb [128, 2048]: per (hf, bw): bank b = 2hf+bw, head h = b+4hh:
    # expb[64hh+k, 1024hf+512bw+64li+q] = exp(bias[h][q, k]), replicated
    # over the 8 lines of a group.
    expb = np.zeros((128, 2048), np.float32)
    for hf in range(2):
        for bw in range(2):
            b = 2 * hf + bw
            for hh in range(2):
                blk = np.exp(relative_bias[b + 4 * hh].T)      # [k, q]
                expb[64 * hh:64 * (hh + 1),
                     1024 * hf + 512 * bw:1024 * hf + 512 * (bw + 1)] = \
                    np.tile(blk, (1, GRP))
    bz = np.zeros((128, 256), np.float32)
    bz[0:64, 0:32] = 1.0
    bz[64:128, 32:64] = 1.0
    bz[0:64, 128 + 64:128 + 96] = 1.0
    bz[64:128, 128 + 96:128 + 128] = 1.0
    bq = np.stack([bqv[:128], bqv[128:]], axis=1).astype(np.float32)
    bpv = bv @ w_proj + b_proj
    bp = np.stack([bpv[:128], bpv[128:]], axis=1).astype(np.float32)
    st = np.zeros((128, ST_COLS), np.float32)
    st[:, 0:512] = wqk_full[:128]
    st[:, 512:1024] = wqk_full[128:]
    st[:, 1024:1280] = wvm[:128]
    st[:, 1280:1536] = wvm[128:]
    for kc in range(2):
        for mc in range(2):
            st[:, 1536 + 256 * kc + 128 * mc:1536 + 256 * kc + 128 * (mc + 1)] = \
                wp_perm[128 * kc:128 * (kc + 1), 128 * mc:128 * (mc + 1)]
    st[:, 2048:4096] = expb
    st[:, 4096:4352] = bz
    st[:, 4352:4354] = bq
    st[:, 4354:4356] = bp
    return dict(statics=st)


LAST_RESULTS = None


def kernel(x, relative_bias, w_qkv, b_qkv, w_proj, b_proj):
    import os
    import sys
    if '/opt/trn_rl_repo' not in sys.path:
        sys.path.insert(0, '/opt/trn_rl_repo')
    from concourse.bass_utils import run_bass_kernel_spmd

    x = np.asarray(x, np.float32)
    const = _host_inputs(x,
                         np.asarray(relative_bias, np.float32),
                         np.asarray(w_qkv, np.float32),
                         np.asarray(b_qkv, np.float32),
                         np.asarray(w_proj, np.float32),
                         np.asarray(b_proj, np.float32))
    xr = np.ascontiguousarray(x.reshape(NBT, C, HW))
    nc = _build_bass()
    in_maps = []
    for c in range(NCORES):
        m = dict(const)
        m["x"] = np.ascontiguousarray(xr[c * BT_PER_CORE:(c + 1) * BT_PER_CORE])
        in_maps.append(m)
    res = run_bass_kernel_spmd(nc, in_maps, list(range(NCORES)),
                               tmpdir=os.environ.get("BASS_TMPDIR"))
    global LAST_RESULTS
    LAST_RESULTS = res
    outs = res.results
    out = np.concatenate([o["out"].reshape(BT_PER_CORE, C, HW) for o in outs],
                         axis=0)
    return out.reshape(B, T, C, H, W).astype(np.float32)
